# revision 1
# baseline (speedup 1.0000x reference)
"""Trainium2 Bass kernel for nn_Attention (dense transformer block).

Strategy: data-parallel over batch across 8 NeuronCores (8 batches/core).
Per core, per batch (N=256 tokens, 16 heads, dim_head=32):
  - qkv projection: q,k computed TRANSPOSED (qkT [j, n], weight-stationary),
    v computed untransposed (x^T-stationary) -> v [n, j] so the attn@v matmul
    needs no on-chip transposes at all.
  - dots^T[m, n] = k_h^T.T-stationary @ q_h^T streaming, K=32, 4 heads packed
    into the 4 PE row-groups (concurrent matmuls).
  - softmax without max-subtraction (|dots| <~ 1.5 by construction) and with
    normalization deferred: attn_unnorm = exp(dots^T) * exp(bias^T) (exp on
    ACT reading PSUM directly, bias multiply on DVE/GPSIMD in bf16).
  - attn@v: out_h^T[d, n] = v_h[m, d]-stationary @ attn^T streaming, 4 heads
    packed into PE col-groups; a parallel ones[m,32]-stationary matmul
    computes the softmax denominators as a 32-row broadcast, so
    reciprocal+normalize are dense per-partition DVE ops.
  - out projection with b_out folded in as a K=1 matmul row; PSUM -> DRAM DMA.
All matmuls in bf16 (fp32 PSUM accumulation); rel-err vs fp32 reference ~1e-3.
"""

import os
import sys

import numpy as np

if "/opt/trn_rl_repo" not in sys.path:
    sys.path.insert(0, "/opt/trn_rl_repo")

import ml_dtypes  # noqa: E402

from concourse import bacc, mybir  # noqa: E402
from concourse.tile import TileContext  # noqa: E402
from concourse.bass_utils import run_bass_kernel_spmd  # noqa: E402

BF16 = mybir.dt.bfloat16
F32 = mybir.dt.float32
NPBF16 = ml_dtypes.bfloat16

B, N, INP, OUP, H, D = 64, 256, 512, 512, 16, 32
NCORES = 8
BL = B // NCORES  # batches per core
SCALE = D ** -0.5

_CACHE = {}


def _relative_index(ih: int, iw: int) -> np.ndarray:
    yy, xx = np.meshgrid(np.arange(ih), np.arange(iw), indexing="ij")
    coords = np.stack([yy.ravel(), xx.ravel()])
    rel = coords[:, :, None] - coords[:, None, :]
    rel[0] += ih - 1
    rel[1] += iw - 1
    rel[0] *= 2 * iw - 1
    return rel.sum(0).ravel()


DEFAULT_OPTS = {
    "bias_dve_mod": 3,      # (2g+mt) % 8 < this -> DVE, else GPSIMD
    "q0_merged_dma": False,  # 4 merged q0 DMAs vs 16 per-head
    "pbig_bufs": 2,
    "pod_bufs": 2,
    "psmall_bufs": 2,
    "xpool_bufs": 3,
    "qkv_bufs": 2,
    "attn_bufs": 2,
    "evac_chunks": 2,
    "v_after_dots": False,
    "small_bufs": 4,
    "q0_bufs": 2,
}


def _build(bl: int, repeats: int = 1, opts: dict | None = None):
    o = dict(DEFAULT_OPTS)
    if opts:
        o.update(opts)
    nc = bacc.Bacc(None, target_bir_lowering=False)

    xT = nc.declare_dram_parameter("xT", [bl, 128, 4, 256], BF16, isOutput=False)
    wqkv = nc.declare_dram_parameter("wqkv", [128, 4, 1536], BF16, isOutput=False)
    w2t = nc.declare_dram_parameter("w2t", [128, 4, 512], BF16, isOutput=False)
    ebT = nc.declare_dram_parameter("ebT", [128, 2, 4096], BF16, isOutput=False)
    bout = nc.declare_dram_parameter("bout", [1, 512], F32, isOutput=False)
    boutb = nc.declare_dram_parameter("boutb", [1, 512], BF16, isOutput=False)
    y = nc.declare_dram_parameter("y", [bl, 2, 128, 512], F32, isOutput=True)

    EXP = mybir.ActivationFunctionType.Exp

    with TileContext(nc) as tc:
        with (
            tc.tile_pool(name="consts", bufs=1) as consts,
            tc.tile_pool(name="xpool", bufs=o["xpool_bufs"]) as xpool,
            tc.tile_pool(name="qkvpool", bufs=o["qkv_bufs"]) as qkvpool,
            tc.tile_pool(name="attnpool", bufs=o["attn_bufs"]) as attnpool,
            tc.tile_pool(name="small", bufs=o["small_bufs"]) as small,
            tc.tile_pool(name="pbig", bufs=o["pbig_bufs"], space="PSUM") as pbig,
            tc.tile_pool(name="pod", bufs=o["pod_bufs"], space="PSUM") as pod,
            tc.tile_pool(name="psmall", bufs=o["psmall_bufs"], space="PSUM") as psmall,
        ):
            # constant loads spread across engine DMA queues so the first
            # batch's x tile (sync queue) isn't stuck behind them
            xt_pre = xpool.tile([128, 4, 256], BF16, tag="xt", name="xt")
            nc.sync.dma_start(xt_pre[:], xT[0])
            wq_sb = consts.tile([128, 4, 1536], BF16)
            nc.sync.dma_start(wq_sb[:, 0, :], wqkv[:, 0, :])
            nc.scalar.dma_start(wq_sb[:, 1, :], wqkv[:, 1, :])
            nc.sync.dma_start(wq_sb[:, 2, :], wqkv[:, 2, :])
            nc.scalar.dma_start(wq_sb[:, 3, :], wqkv[:, 3, :])
            w2_sb = consts.tile([128, 4, 512], BF16)
            nc.scalar.dma_start(w2_sb[:], w2t[:])
            eb_sb = consts.tile([128, 2, 4096], BF16)
            nc.gpsimd.dma_start(eb_sb[:], ebT[:])
            bout_bc = consts.tile([128, 512], F32)
            nc.scalar.dma_start(bout_bc[:], bout[:].to_broadcast((128, 512)))
            bo_sb = consts.tile([1, 512], BF16)
            nc.scalar.dma_start(bo_sb[:], boutb[:])
            ones32 = consts.tile([128, 32], BF16)
            nc.vector.memset(ones32[:], 1.0)
            ones1 = consts.tile([1, 128], BF16)
            nc.vector.memset(ones1[:], 1.0)
            # zero-padded q staging: q0[p, h, n] nonzero only for
            # p in [32*(h%4), 32*(h%4)+32); the zero rows are written once
            # and never touched again (per-batch DMAs overwrite only the
            # nonzero rows), so the cross-head terms of the full-K dots
            # matmuls vanish. Two buffers, alternated by batch parity.
            q0 = []
            for i in range(o["q0_bufs"]):
                t = consts.tile([128, 16, 256], BF16, name=f"q0_{i}")
                nc.vector.memset(t[:], 0.0)
                q0.append(t)

            for rep in range(repeats):
              for b in range(bl):
                if rep == 0 and b == 0:
                    xt = xt_pre
                else:
                    xt = xpool.tile([128, 4, 256], BF16, tag="xt", name="xt")
                    nc.sync.dma_start(xt[:], xT[b])

                qkT = qkvpool.tile([128, 2048], BF16, tag="qkT")
                vt = qkvpool.tile([128, 2, 512], BF16, tag="vt")

                # q,k projection (transposed): out[j, n] over j-tiles 0..7
                for half in range(2):
                    pqk = pbig.tile([128, 1024], F32, tag="pqk", bufs=1, name="pqk")
                    for jq in range(4):
                        jt = half * 4 + jq
                        for it in range(4):
                            nc.tensor.matmul(
                                pqk[:, jq * 256 : (jq + 1) * 256],
                                lhsT=wq_sb[:, it, jt * 128 : (jt + 1) * 128],
                                rhs=xt[:, it, :],
                                start=(it == 0),
                                stop=(it == 3),
                            )
                    if o["evac_chunks"] == 1 or half == 1:
                        nc.vector.tensor_copy(
                            out=qkT[:, half * 1024 : (half + 1) * 1024], in_=pqk[:]
                        )
                    else:
                        ch = 1024 // o["evac_chunks"]
                        for ci in range(o["evac_chunks"]):
                            nc.vector.tensor_copy(
                                out=qkT[:, ci * ch : (ci + 1) * ch],
                                in_=pqk[:, ci * ch : (ci + 1) * ch],
                            )

                def emit_v():
                    # v projection (untransposed): v[n, j]
                    for nt in range(2):
                        pv = pod.tile([128, 512], F32, tag="pod", name="pv")
                        for it in range(4):
                            nc.tensor.matmul(
                                pv[:],
                                lhsT=xt[:, it, nt * 128 : (nt + 1) * 128],
                                rhs=wq_sb[:, it, 1024:1536],
                                start=(it == 0),
                                stop=(it == 3),
                            )
                        nc.vector.tensor_copy(out=vt[:, nt, :], in_=pv[:])

                if not o["v_after_dots"]:
                    emit_v()

                # stage zero-padded q tiles (pure DMA, no engine cost);
                # one DMA per hp covers all four groups g: head h = 4g+hp
                # lives at partitions [32hp, 32hp+32), dst slot h, src block g.
                qz = q0[b % o["q0_bufs"]]
                if o["q0_merged_dma"] == "8way":
                    qz_g = qz.rearrange("p (g q) n -> p g q n", q=4)
                    for hp in range(4):
                        for gh in range(2):
                            nc.sync.dma_start(
                                out=qz_g[
                                    32 * hp : 32 * (hp + 1), 2 * gh : 2 * gh + 2, hp, :
                                ],
                                in_=qkT[
                                    32 * hp : 32 * (hp + 1),
                                    512 * gh : 512 * (gh + 1),
                                ].rearrange("p (g n) -> p g n", n=256),
                            )
                elif o["q0_merged_dma"]:
                    qz_g = qz.rearrange("p (g q) n -> p g q n", q=4)
                    for hp in range(4):
                        nc.sync.dma_start(
                            out=qz_g[32 * hp : 32 * (hp + 1), :, hp, :],
                            in_=qkT[32 * hp : 32 * (hp + 1), 0:1024].rearrange(
                                "p (g n) -> p g n", n=256
                            ),
                        )
                else:
                    for h in range(H):
                        hp, g = h % 4, h // 4
                        nc.sync.dma_start(
                            out=qz[32 * hp : 32 * (hp + 1), h, :],
                            in_=qkT[32 * hp : 32 * (hp + 1), g * 256 : (g + 1) * 256],
                        )

                # attention scores, exp, bias
                attn = [
                    attnpool.tile(
                        [128, 4096], BF16, tag=f"attn{mt}", name=f"attn{mt}"
                    )
                    for mt in range(2)
                ]
                for g in range(4):
                    for mt in range(2):
                        pd = pbig.tile([128, 1024], F32, tag="pbig")
                        for t in range(2):
                            nc.tensor.matmul(
                                pd[:, t * 512 : (t + 1) * 512],
                                lhsT=qkT[
                                    :,
                                    (4 + g) * 256 + mt * 128 : (4 + g) * 256
                                    + (mt + 1) * 128,
                                ],
                                rhs=qz[:, 4 * g + 2 * t : 4 * g + 2 * t + 2, :],
                                start=True,
                                stop=True,
                            )
                        nc.scalar.activation(
                            out=attn[mt][:, g * 1024 : (g + 1) * 1024],
                            in_=pd[:],
                            func=EXP,
                        )
                        eng = nc.vector if (2 * g + mt) % 8 < o["bias_dve_mod"] else nc.gpsimd
                        eng.tensor_mul(
                            attn[mt][:, g * 1024 : (g + 1) * 1024],
                            attn[mt][:, g * 1024 : (g + 1) * 1024],
                            eb_sb[:, mt, g * 1024 : (g + 1) * 1024],
                        )

                if o["v_after_dots"]:
                    emit_v()

                # attn @ v (+ denominators via ones-stationary matmuls)
                outT = small.tile([128, 1024], BF16, tag="outT")
                for g in range(4):
                    od = pod.tile([128, 512], F32, tag="pod")
                    # mt-outer so the four col-group matmuls issue
                    # back-to-back (enables per-subarray concurrency)
                    for mt in range(2):
                        for hp in range(4):
                            h = 4 * g + hp
                            nc.tensor.matmul(
                                od[32 * hp : 32 * (hp + 1), 0:256],
                                lhsT=vt[:, mt, 32 * h : 32 * h + 32],
                                rhs=attn[mt][:, h * 256 : (h + 1) * 256],
                                start=(mt == 0),
                                stop=(mt == 1),
                                tile_position=(0, 32 * hp),
                                skip_group_check=True,
                            )
                    for mt in range(2):
                        for hp in range(4):
                            h = 4 * g + hp
                            nc.tensor.matmul(
                                od[32 * hp : 32 * (hp + 1), 256:512],
                                lhsT=ones32[:],
                                rhs=attn[mt][:, h * 256 : (h + 1) * 256],
                                start=(mt == 0),
                                stop=(mt == 1),
                                tile_position=(0, 32 * hp),
                                skip_group_check=True,
                            )
                    r = small.tile([128, 256], F32, tag="r")
                    nc.vector.reciprocal_approx_fast(out=r[:], in_=od[:, 256:512])
                    nc.vector.tensor_mul(
                        outT[:, g * 256 : (g + 1) * 256], od[:, 0:256], r[:]
                    )

                # output projection + bias, straight to DRAM from PSUM
                for nt in range(2):
                    py = pod.tile([128, 512], F32, tag="pod", name="py")
                    for ot in range(4):
                        nc.tensor.matmul(
                            py[:],
                            lhsT=outT[
                                :, ot * 256 + nt * 128 : ot * 256 + nt * 128 + 128
                            ],
                            rhs=w2_sb[:, ot, :],
                            start=(ot == 0),
                            stop=(ot == 3) and nt == 0,
                        )
                    if nt == 1:
                        nc.tensor.matmul(
                            py[:], lhsT=ones1[:], rhs=bo_sb[:], start=False,
                            stop=True,
                        )
                    ysb = small.tile([128, 512], F32, tag="ysb", name="ysb")
                    if nt == 0:
                        nc.vector.tensor_add(ysb[:], py[:], bout_bc[:])
                    else:
                        nc.scalar.copy(out=ysb[:], in_=py[:])
                    nc.sync.dma_start(out=y[b, nt], in_=ysb[:])

    nc.compile()
    return nc


def _get_nc(bl: int, repeats: int = 1, opts: dict | None = None):
    key = (bl, repeats, tuple(sorted((opts or {}).items())))
    if key not in _CACHE:
        _CACHE[key] = _build(bl, repeats, opts)
    return _CACHE[key]


def _prep_inputs(x, w_qkv, rel_bias_table, w_out, b_out):
    """Host-side layout prep: transpose/tile/bf16-cast, bias-table gather."""
    x = np.asarray(x, np.float32)
    w_qkv = np.asarray(w_qkv, np.float32).copy()
    rel_bias_table = np.asarray(rel_bias_table, np.float32)
    w_out = np.asarray(w_out, np.float32)
    b_out = np.asarray(b_out, np.float32)

    # fold the attention scale into the q columns of w_qkv
    w_qkv[:, :OUP] *= SCALE

    # xT_dev[b, p, it, n] = x[b, n, it*128+p]
    xT = np.ascontiguousarray(
        x.transpose(0, 2, 1).reshape(B, 4, 128, N).transpose(0, 2, 1, 3)
    ).astype(NPBF16)
    # wqkv_dev[p, it, j] = w_qkv[it*128+p, j]
    wqkv_dev = np.ascontiguousarray(
        w_qkv.reshape(4, 128, 3 * OUP).transpose(1, 0, 2)
    ).astype(NPBF16)
    # w2t_dev[p, ot, q] = w_out.T[ot*128+p, q] = w_out[q, ot*128+p]
    w2t_dev = np.ascontiguousarray(
        w_out.T.reshape(4, 128, OUP).transpose(1, 0, 2)
    ).astype(NPBF16)
    # bias[n, m, h]; ebT_dev[p, mt, h*256+n] = exp(bias[n, mt*128+p, h])
    rel_idx = _relative_index(16, 16)
    bias = rel_bias_table[rel_idx].reshape(N, N, H)  # [n, m, h]
    ebT = np.exp(bias.transpose(2, 1, 0))  # [h, m, n]
    ebT_dev = np.ascontiguousarray(
        ebT.reshape(H, 2, 128, N).transpose(2, 1, 0, 3).reshape(128, 2, H * N)
    ).astype(NPBF16)
    bout_dev = b_out.reshape(1, OUP).astype(np.float32)
    return xT, wqkv_dev, w2t_dev, ebT_dev, bout_dev


def kernel(x, w_qkv, rel_bias_table, w_out, b_out, ih, iw):
    assert int(ih) == 16 and int(iw) == 16
    xT, wqkv_dev, w2t_dev, ebT_dev, bout_dev = _prep_inputs(
        x, w_qkv, rel_bias_table, w_out, b_out
    )

    nc = _get_nc(BL)
    in_maps = []
    for c in range(NCORES):
        in_maps.append(
            {
                "xT": np.ascontiguousarray(xT[c * BL : (c + 1) * BL]),
                "wqkv": wqkv_dev,
                "w2t": w2t_dev,
                "ebT": ebT_dev,
                "bout": bout_dev,
                "boutb": bout_dev.astype(NPBF16),
            }
        )

    trace = bool(os.environ.get("BASS_TRACE_KERNEL"))
    if trace:
        try:
            from antenv.axon_hooks import get_axon_ntff_profile_hook  # noqa: F401
        except ImportError:
            trace = False
    res = run_bass_kernel_spmd(nc, in_maps, core_ids=list(range(NCORES)), trace=trace)
    kernel.last_result = res
    if res.exec_time_ns is not None:
        print(f"HW exec time: {res.exec_time_ns} ns")

    y = np.concatenate(
        [r["y"].reshape(BL, N, OUP) for r in res.results], axis=0
    ).astype(np.float32)
    return y


kernel.last_result = None



# revision 4
# speedup vs baseline: 1.0326x; 1.0326x over previous
"""Trainium2 Bass kernel for nn_Attention — v1 rewrite.

Data-parallel over batch across 8 NeuronCores (8 batches/core, processed in
pairs). Per core:
  - qkv projection batch-PAIR weight-stationary: rhs = 2 batches' tokens
    (N=512 streams, halves LDWEIGHTS on HW); q,k come out transposed
    (qkT [j, (b,n)]), v untransposed (v [n, j]).
  - dots^T[m, n] per head via K=32 ROW-PACKED matmuls (tile_position row
    groups): lhsT = k_h^T slice, rhs = q_h^T slice read DIRECTLY from qkT —
    no zero-padded q staging, no SBUF->SBUF DMAs. 4 heads of a group run in
    4 row groups concurrently on HW.
  - softmax without max-subtraction, normalization deferred:
    attn = exp(dots^T) * exp(bias^T) (exp on ACT from PSUM, bias mul on
    DVE/GPSIMD in bf16).
  - attn@v: out_h^T[d, n] = v_h-stationary @ attn^T, 4 heads packed into PE
    col-groups; parallel ones-stationary matmuls give softmax denominators
    as a 32-row broadcast; reciprocal+normalize dense per-partition DVE ops.
  - out projection; bias added via DVE tensor_add from a broadcast tile;
    PSUM -> SBUF -> DRAM.
All matmuls bf16 (fp32 PSUM accumulation).
"""

import os
import sys

import numpy as np

if "/opt/trn_rl_repo" not in sys.path:
    sys.path.insert(0, "/opt/trn_rl_repo")

import ml_dtypes  # noqa: E402

from concourse import bacc, mybir  # noqa: E402
from concourse.tile import TileContext  # noqa: E402
from concourse.bass_utils import run_bass_kernel_spmd  # noqa: E402

BF16 = mybir.dt.bfloat16
F32 = mybir.dt.float32
NPBF16 = ml_dtypes.bfloat16

B, N, INP, OUP, H, D = 64, 256, 512, 512, 16, 32
NCORES = 8
BL = B // NCORES  # batches per core
SCALE = D ** -0.5

_CACHE = {}


def _relative_index(ih: int, iw: int) -> np.ndarray:
    yy, xx = np.meshgrid(np.arange(ih), np.arange(iw), indexing="ij")
    coords = np.stack([yy.ravel(), xx.ravel()])
    rel = coords[:, :, None] - coords[:, None, :]
    rel[0] += ih - 1
    rel[1] += iw - 1
    rel[0] *= 2 * iw - 1
    return rel.sum(0).ravel()


DEFAULT_OPTS = {
    "eb_dve_mod": 0,        # (2g+mt) % 8 < this -> DVE, else GPSIMD
    "pd_bufs": 1,
    "pod_bufs": 2,
    "podd_bufs": 2,
    "xpool_bufs": 3,
    "qkv_bufs": 2,
    "vt_bufs": 4,
    "attn_bufs": 2,
    "small_bufs": 4,
    "qk_evac_act": 0,       # how many of the 8 per-pair qk evacs go to ACT
    "v_evac_act": False,
    "warmup_mms": 8,
}


def _build(bl: int, repeats: int = 1, opts: dict | None = None):
    o = dict(DEFAULT_OPTS)
    if opts:
        o.update(opts)
    nc = bacc.Bacc(None, target_bir_lowering=False)
    npairs = bl // 2

    # xT[bp, p, it, b*256+n] = x[2bp+b, n, it*128+p]
    xT = nc.declare_dram_parameter("xT", [npairs, 128, 4, 512], BF16, isOutput=False)
    wqkv = nc.declare_dram_parameter("wqkv", [128, 4, 1536], BF16, isOutput=False)
    w2t = nc.declare_dram_parameter("w2t", [128, 4, 512], BF16, isOutput=False)
    ebT = nc.declare_dram_parameter("ebT", [128, 2, 4096], BF16, isOutput=False)
    bout = nc.declare_dram_parameter("bout", [1, 512], F32, isOutput=False)
    y = nc.declare_dram_parameter("y", [bl, 2, 128, 512], F32, isOutput=True)

    EXP = mybir.ActivationFunctionType.Exp

    with TileContext(nc) as tc:
        with (
            tc.tile_pool(name="consts", bufs=1) as consts,
            tc.tile_pool(name="xpool", bufs=o["xpool_bufs"]) as xpool,
            tc.tile_pool(name="qkvpool", bufs=o["qkv_bufs"]) as qkvpool,
            tc.tile_pool(name="vtpool", bufs=o["vt_bufs"]) as vtpool,
            tc.tile_pool(name="attnpool", bufs=o["attn_bufs"]) as attnpool,
            tc.tile_pool(name="small", bufs=o["small_bufs"]) as small,
            tc.tile_pool(name="pd", bufs=o["pd_bufs"], space="PSUM") as pdp,
            tc.tile_pool(name="pod", bufs=o["pod_bufs"], space="PSUM") as pod,
            tc.tile_pool(name="podd", bufs=o["podd_bufs"], space="PSUM") as podd,
        ):
            # first x pair on the sync queue ahead of consts so batch 0 can
            # start; qkv weights spread across four engine DMA queues so
            # they all land in parallel
            xt_pre = xpool.tile([128, 4, 512], BF16, tag="xt", name="xt")
            nc.sync.dma_start(xt_pre[:], xT[0])
            wq_sb = consts.tile([128, 4, 1536], BF16)
            nc.scalar.dma_start(wq_sb[:, 0, :], wqkv[:, 0, :])
            nc.gpsimd.dma_start(wq_sb[:, 1, :], wqkv[:, 1, :])
            nc.scalar.dma_start(wq_sb[:, 2, :], wqkv[:, 2, :])
            nc.sync.dma_start(wq_sb[:, 3, :], wqkv[:, 3, :])
            w2_sb = consts.tile([128, 4, 512], BF16)
            nc.scalar.dma_start(w2_sb[:], w2t[:])
            eb_sb = consts.tile([128, 2, 4096], BF16)
            nc.gpsimd.dma_start(eb_sb[:], ebT[:])
            bout_bc = consts.tile([128, 512], F32)
            nc.scalar.dma_start(bout_bc[:], bout[:].to_broadcast((128, 512)))
            ones32 = consts.tile([128, 32], BF16)
            nc.vector.memset(ones32[:], 1.0)
            # warm the PE clock during the initial DMA wait: ~3us of dummy
            # matmuls on a memset tile so the first real matmuls run at
            # full clock (mirrors the HAM warmup window on HW)
            if o["warmup_mms"]:
                wrm = consts.tile([128, 512], BF16)
                nc.vector.memset(wrm[:], 0.0)
                pw = pod.tile([128, 512], F32, tag="pod", name="pwarm")
                for i in range(o["warmup_mms"]):
                    nc.tensor.matmul(
                        pw[:],
                        lhsT=wrm[:, 0:128],
                        rhs=wrm[:],
                        start=(i == 0),
                        stop=(i == o["warmup_mms"] - 1),
                    )

            pair_data = {}

            def gen_qkv(bp, xt=None):
                """Emit qkv projection for pair bp; yields between groups."""
                if xt is None:
                    xt = xpool.tile([128, 4, 512], BF16, tag="xt", name="xt")
                    nc.sync.dma_start(xt[:], xT[bp % npairs])
                # q,k projection (transposed), pair-wide: out[j, (b,n)]
                qkT = qkvpool.tile([128, 8, 512], BF16, tag="qkT")
                vts = []
                pair_data[bp] = (qkT, vts)
                for jt in range(8):
                    pqk = pod.tile([128, 512], F32, tag="pod", name="pqk")
                    for it in range(4):
                        nc.tensor.matmul(
                            pqk[:],
                            lhsT=wq_sb[:, it, jt * 128 : (jt + 1) * 128],
                            rhs=xt[:, it, :],
                            start=(it == 0),
                            stop=(it == 3),
                        )
                    if jt < o["qk_evac_act"]:
                        nc.scalar.copy(out=qkT[:, jt, :], in_=pqk[:])
                    else:
                        nc.vector.tensor_copy(out=qkT[:, jt, :], in_=pqk[:])
                    yield
                # v projection (untransposed), per batch: v[n, j]
                for b in range(2):
                    vt = vtpool.tile([128, 2, 512], BF16, tag="vt")
                    for nt in range(2):
                        pv = pod.tile([128, 512], F32, tag="pod", name="pv")
                        for it in range(4):
                            nc.tensor.matmul(
                                pv[:],
                                lhsT=xt[:, it, b * 256 + nt * 128 : b * 256 + nt * 128 + 128],
                                rhs=wq_sb[:, it, 1024:1536],
                                start=(it == 0),
                                stop=(it == 3),
                            )
                        if o["v_evac_act"]:
                            nc.scalar.copy(out=vt[:, nt, :], in_=pv[:])
                        else:
                            nc.vector.tensor_copy(out=vt[:, nt, :], in_=pv[:])
                        yield
                    vts.append(vt)

            def gen_attn(bp):
                """Emit attention+output for pair bp; yields between groups."""
                qkT, vts = pair_data[bp]
                for b in range(2):
                    vt = vts[b]
                    # attention scores via K=32 row-packed matmuls, exp, bias
                    attn = [
                        attnpool.tile([128, 4096], BF16, tag=f"attn{mt}",
                                      name=f"attn{mt}")
                        for mt in range(2)
                    ]
                    for g in range(4):
                        for mt in range(2):
                            # 4 row-group matmuls run concurrently on HW and
                            # drain to the SAME partitions — each must target
                            # its own PSUM bank (same-bank row-packed writes
                            # are a fatal PSUM collision). Head hp writes the
                            # first half of bank hp of a 4-bank tile.
                            pd = pdp.tile([128, 2048], F32, tag="pd")
                            for hp in range(4):
                                nc.tensor.matmul(
                                    pd[:, hp * 512 : hp * 512 + 256],
                                    lhsT=qkT[
                                        32 * hp : 32 * (hp + 1),
                                        4 + g,
                                        b * 256 + mt * 128 : b * 256 + mt * 128 + 128,
                                    ],
                                    rhs=qkT[
                                        32 * hp : 32 * (hp + 1),
                                        g,
                                        b * 256 : (b + 1) * 256,
                                    ],
                                    start=True,
                                    stop=True,
                                    tile_position=(32 * hp, 0),
                                    skip_group_check=True,
                                )
                            nc.scalar.activation(
                                out=attn[mt][
                                    :, g * 1024 : (g + 1) * 1024
                                ].rearrange("p (q n) -> p q n", n=256),
                                in_=pd[:].rearrange("p (q n) -> p q n", n=512)[
                                    :, :, 0:256
                                ],
                                func=EXP,
                            )
                            eng = (
                                nc.vector
                                if (2 * g + mt) % 8 < o["eb_dve_mod"]
                                else nc.gpsimd
                            )
                            eng.tensor_mul(
                                attn[mt][:, g * 1024 : (g + 1) * 1024],
                                attn[mt][:, g * 1024 : (g + 1) * 1024],
                                eb_sb[:, mt, g * 1024 : (g + 1) * 1024],
                            )
                            yield "dots"

                    # attn @ v (+ denominators via ones-stationary matmuls)
                    outT = small.tile([128, 1024], BF16, tag="outT")
                    for g in range(4):
                        od = podd.tile([128, 512], F32, tag="podd")
                        # mt-outer so the four col-group matmuls issue
                        # back-to-back (per-subarray concurrency)
                        for mt in range(2):
                            for hp in range(4):
                                h = 4 * g + hp
                                nc.tensor.matmul(
                                    od[32 * hp : 32 * (hp + 1), 0:256],
                                    lhsT=vt[:, mt, 32 * h : 32 * h + 32],
                                    rhs=attn[mt][:, h * 256 : (h + 1) * 256],
                                    start=(mt == 0),
                                    stop=(mt == 1),
                                    tile_position=(0, 32 * hp),
                                    skip_group_check=True,
                                )
                        for mt in range(2):
                            for hp in range(4):
                                h = 4 * g + hp
                                nc.tensor.matmul(
                                    od[32 * hp : 32 * (hp + 1), 256:512],
                                    lhsT=ones32[:],
                                    rhs=attn[mt][:, h * 256 : (h + 1) * 256],
                                    start=(mt == 0),
                                    stop=(mt == 1),
                                    tile_position=(0, 32 * hp),
                                    skip_group_check=True,
                                )
                        r = small.tile([128, 256], F32, tag="r")
                        nc.vector.reciprocal_approx_fast(out=r[:], in_=od[:, 256:512])
                        nc.vector.tensor_mul(
                            outT[:, g * 256 : (g + 1) * 256], od[:, 0:256], r[:]
                        )
                        yield "attnv"

                    # output projection + bias, PSUM -> SBUF -> DRAM.
                    # Deferred: emitted later as PE filler during the next
                    # batch's ACT-bound dots phase.
                    def emit_proj(b, outT):
                        for nt in range(2):
                            py = pod.tile([128, 512], F32, tag="pod", name="py")
                            for ot in range(4):
                                nc.tensor.matmul(
                                    py[:],
                                    lhsT=outT[
                                        :,
                                        ot * 256 + nt * 128 : ot * 256 + nt * 128 + 128,
                                    ],
                                    rhs=w2_sb[:, ot, :],
                                    start=(ot == 0),
                                    stop=(ot == 3),
                                )
                            ysb = small.tile([128, 512], F32, tag="ysb", name="ysb")
                            nc.vector.tensor_add(ysb[:], py[:], bout_bc[:])
                            nc.sync.dma_start(
                                out=y[(2 * bp + b) % bl, nt], in_=ysb[:]
                            )
                            yield "proj"
                    yield ("proj_gen", emit_proj(b, outT))

            # software pipeline: qkv(pair p+1) groups and deferred output
            # projections are interleaved into the ACT-bound dots phase of
            # attention(pair p) so the PE FIFO never head-of-line blocks on
            # the exp drain.
            total_pairs = repeats * npairs
            for _ in gen_qkv(0, xt=xt_pre):
                pass
            _SENTINEL = object()
            fillers = []

            def spend_one():
                while fillers:
                    if next(fillers[0], _SENTINEL) is _SENTINEL:
                        fillers.pop(0)
                        continue
                    return True
                return False

            for p in range(total_pairs):
                if p + 1 < total_pairs:
                    fillers.append(gen_qkv(p + 1))
                credit = 0.0
                for item in gen_attn(p):
                    if isinstance(item, tuple) and item[0] == "proj_gen":
                        fillers.append(item[1])
                        continue
                    if item == "dots":
                        credit += 1.0
                        while credit >= 1.0 and spend_one():
                            credit -= 1.0
                # flush before the next pipeline stage
                while spend_one():
                    pass
                del pair_data[p]

    nc.compile()
    return nc


def _get_nc(bl: int, repeats: int = 1, opts: dict | None = None):
    key = (bl, repeats, tuple(sorted((opts or {}).items())))
    if key not in _CACHE:
        _CACHE[key] = _build(bl, repeats, opts)
    return _CACHE[key]


def _prep_inputs(x, w_qkv, rel_bias_table, w_out, b_out):
    """Host-side layout prep: transpose/tile/bf16-cast, bias-table gather."""
    x = np.asarray(x, np.float32)
    w_qkv = np.asarray(w_qkv, np.float32).copy()
    rel_bias_table = np.asarray(rel_bias_table, np.float32)
    w_out = np.asarray(w_out, np.float32)
    b_out = np.asarray(b_out, np.float32)

    # fold the attention scale into the q columns of w_qkv
    w_qkv[:, :OUP] *= SCALE

    # xT_dev[bp, p, it, b*256+n] = x[2bp+b, n, it*128+p]
    xT = (
        x.transpose(0, 2, 1)                 # [B, inp, n]
        .reshape(B // 2, 2, 4, 128, N)       # [bp, b, it, p, n]
        .transpose(0, 3, 2, 1, 4)            # [bp, p, it, b, n]
        .reshape(B // 2, 128, 4, 2 * N)
    )
    xT = np.ascontiguousarray(xT).astype(NPBF16)
    # wqkv_dev[p, it, j] = w_qkv[it*128+p, j]
    wqkv_dev = np.ascontiguousarray(
        w_qkv.reshape(4, 128, 3 * OUP).transpose(1, 0, 2)
    ).astype(NPBF16)
    # w2t_dev[p, ot, q] = w_out.T[ot*128+p, q] = w_out[q, ot*128+p]
    w2t_dev = np.ascontiguousarray(
        w_out.T.reshape(4, 128, OUP).transpose(1, 0, 2)
    ).astype(NPBF16)
    # bias[n, m, h]; ebT_dev[p, mt, h*256+n] = exp(bias[n, mt*128+p, h])
    rel_idx = _relative_index(16, 16)
    bias = rel_bias_table[rel_idx].reshape(N, N, H)  # [n, m, h]
    ebT = np.exp(bias.transpose(2, 1, 0))  # [h, m, n]
    ebT_dev = np.ascontiguousarray(
        ebT.reshape(H, 2, 128, N).transpose(2, 1, 0, 3).reshape(128, 2, H * N)
    ).astype(NPBF16)
    bout_dev = b_out.reshape(1, OUP).astype(np.float32)
    return xT, wqkv_dev, w2t_dev, ebT_dev, bout_dev


def kernel(x, w_qkv, rel_bias_table, w_out, b_out, ih, iw):
    assert int(ih) == 16 and int(iw) == 16
    xT, wqkv_dev, w2t_dev, ebT_dev, bout_dev = _prep_inputs(
        x, w_qkv, rel_bias_table, w_out, b_out
    )

    nc = _get_nc(BL)
    npairs = BL // 2
    in_maps = []
    for c in range(NCORES):
        in_maps.append(
            {
                "xT": np.ascontiguousarray(xT[c * npairs : (c + 1) * npairs]),
                "wqkv": wqkv_dev,
                "w2t": w2t_dev,
                "ebT": ebT_dev,
                "bout": bout_dev,
            }
        )

    trace = bool(os.environ.get("BASS_TRACE_KERNEL"))
    if trace:
        try:
            from antenv.axon_hooks import get_axon_ntff_profile_hook  # noqa: F401
        except ImportError:
            trace = False
    res = run_bass_kernel_spmd(nc, in_maps, core_ids=list(range(NCORES)), trace=trace)
    kernel.last_result = res
    if res.exec_time_ns is not None:
        print(f"HW exec time: {res.exec_time_ns} ns")

    y = np.concatenate(
        [r["y"].reshape(BL, N, OUP) for r in res.results], axis=0
    ).astype(np.float32)
    return y


kernel.last_result = None


# revision 5
# speedup vs baseline: 1.0370x; 1.0042x over previous
"""Trainium2 Bass kernel for nn_Attention — v1 rewrite.

Data-parallel over batch across 8 NeuronCores (8 batches/core, processed in
pairs). Per core:
  - qkv projection batch-PAIR weight-stationary: rhs = 2 batches' tokens
    (N=512 streams, halves LDWEIGHTS on HW); q,k come out transposed
    (qkT [j, (b,n)]), v untransposed (v [n, j]).
  - dots^T[m, n] per head via K=32 ROW-PACKED matmuls (tile_position row
    groups): lhsT = k_h^T slice, rhs = q_h^T slice read DIRECTLY from qkT —
    no zero-padded q staging, no SBUF->SBUF DMAs. 4 heads of a group run in
    4 row groups concurrently on HW.
  - softmax without max-subtraction, normalization deferred:
    attn = exp(dots^T) * exp(bias^T) (exp on ACT from PSUM, bias mul on
    DVE/GPSIMD in bf16).
  - attn@v: out_h^T[d, n] = v_h-stationary @ attn^T, 4 heads packed into PE
    col-groups; parallel ones-stationary matmuls give softmax denominators
    as a 32-row broadcast; reciprocal+normalize dense per-partition DVE ops.
  - out projection; bias added via DVE tensor_add from a broadcast tile;
    PSUM -> SBUF -> DRAM.
All matmuls bf16 (fp32 PSUM accumulation).
"""

import os
import sys

import numpy as np

if "/opt/trn_rl_repo" not in sys.path:
    sys.path.insert(0, "/opt/trn_rl_repo")

import ml_dtypes  # noqa: E402

from concourse import bacc, mybir  # noqa: E402
from concourse.tile import TileContext  # noqa: E402
from concourse.bass_utils import run_bass_kernel_spmd  # noqa: E402

BF16 = mybir.dt.bfloat16
F32 = mybir.dt.float32
NPBF16 = ml_dtypes.bfloat16

B, N, INP, OUP, H, D = 64, 256, 512, 512, 16, 32
NCORES = 8
BL = B // NCORES  # batches per core
SCALE = D ** -0.5

_CACHE = {}


def _relative_index(ih: int, iw: int) -> np.ndarray:
    yy, xx = np.meshgrid(np.arange(ih), np.arange(iw), indexing="ij")
    coords = np.stack([yy.ravel(), xx.ravel()])
    rel = coords[:, :, None] - coords[:, None, :]
    rel[0] += ih - 1
    rel[1] += iw - 1
    rel[0] *= 2 * iw - 1
    return rel.sum(0).ravel()


DEFAULT_OPTS = {
    "eb_dve_mod": 0,        # (2g+mt) % 8 < this -> DVE, else GPSIMD
    "pd_bufs": 1,
    "pod_bufs": 2,
    "podd_bufs": 2,
    "xpool_bufs": 3,
    "qkv_bufs": 2,
    "vt_bufs": 4,
    "attn_bufs": 2,
    "small_bufs": 4,
    "qk_evac_act": 0,       # how many of the 8 per-pair qk evacs go to ACT
    "v_evac_act": False,
    "warmup_mms": 8,
}


def _build(bl: int, repeats: int = 1, opts: dict | None = None):
    o = dict(DEFAULT_OPTS)
    if opts:
        o.update(opts)
    nc = bacc.Bacc(None, target_bir_lowering=False)
    npairs = bl // 2

    # xT[bp, p, it, b*256+n] = x[2bp+b, n, it*128+p]
    xT = nc.declare_dram_parameter("xT", [npairs, 128, 4, 512], BF16, isOutput=False)
    wqkv = nc.declare_dram_parameter("wqkv", [128, 4, 1536], BF16, isOutput=False)
    w2t = nc.declare_dram_parameter("w2t", [128, 4, 512], BF16, isOutput=False)
    ebT = nc.declare_dram_parameter("ebT", [128, 2, 4096], BF16, isOutput=False)
    bout = nc.declare_dram_parameter("bout", [1, 512], F32, isOutput=False)
    y = nc.declare_dram_parameter("y", [bl, 2, 128, 512], F32, isOutput=True)

    EXP = mybir.ActivationFunctionType.Exp

    with TileContext(nc) as tc:
        with (
            tc.tile_pool(name="consts", bufs=1) as consts,
            tc.tile_pool(name="xpool", bufs=o["xpool_bufs"]) as xpool,
            tc.tile_pool(name="qkvpool", bufs=o["qkv_bufs"]) as qkvpool,
            tc.tile_pool(name="vtpool", bufs=o["vt_bufs"]) as vtpool,
            tc.tile_pool(name="attnpool", bufs=o["attn_bufs"]) as attnpool,
            tc.tile_pool(name="small", bufs=o["small_bufs"]) as small,
            tc.tile_pool(name="pd", bufs=o["pd_bufs"], space="PSUM") as pdp,
            tc.tile_pool(name="pod", bufs=o["pod_bufs"], space="PSUM") as pod,
            tc.tile_pool(name="podd", bufs=o["podd_bufs"], space="PSUM") as podd,
        ):
            # first x pair on the sync queue ahead of consts so batch 0 can
            # start; qkv weights spread across four engine DMA queues so
            # they all land in parallel
            xt_pre = xpool.tile([128, 4, 512], BF16, tag="xt", name="xt")
            nc.sync.dma_start(xt_pre[:], xT[0])
            wq_sb = consts.tile([128, 4, 1536], BF16)
            nc.scalar.dma_start(wq_sb[:, 0, :], wqkv[:, 0, :])
            nc.gpsimd.dma_start(wq_sb[:, 1, :], wqkv[:, 1, :])
            nc.scalar.dma_start(wq_sb[:, 2, :], wqkv[:, 2, :])
            nc.sync.dma_start(wq_sb[:, 3, :], wqkv[:, 3, :])
            w2_sb = consts.tile([128, 4, 512], BF16)
            nc.scalar.dma_start(w2_sb[:], w2t[:])
            eb_sb = consts.tile([128, 2, 4096], BF16)
            nc.gpsimd.dma_start(eb_sb[:], ebT[:])
            bout_bc = consts.tile([128, 512], F32)
            nc.scalar.dma_start(bout_bc[:], bout[:].to_broadcast((128, 512)))
            ones32 = consts.tile([128, 32], BF16)
            nc.vector.memset(ones32[:], 1.0)
            # warm the PE clock during the initial DMA wait: ~3us of dummy
            # matmuls on a memset tile so the first real matmuls run at
            # full clock (mirrors the HAM warmup window on HW)
            if o["warmup_mms"]:
                wrm = consts.tile([128, 512], BF16)
                nc.vector.memset(wrm[:], 0.0)
                pw = pod.tile([128, 512], F32, tag="pod", name="pwarm")
                for i in range(o["warmup_mms"]):
                    nc.tensor.matmul(
                        pw[:],
                        lhsT=wrm[:, 0:128],
                        rhs=wrm[:],
                        start=(i == 0),
                        stop=(i == o["warmup_mms"] - 1),
                    )

            pair_data = {}

            def gen_qk(bp, xt=None):
                """Emit q,k projection for pair bp; yields between groups."""
                if xt is None:
                    xt = xpool.tile([128, 4, 512], BF16, tag="xt", name="xt")
                    nc.sync.dma_start(xt[:], xT[bp % npairs])
                # q,k projection (transposed), pair-wide: out[j, (b,n)]
                qkT = qkvpool.tile([128, 8, 512], BF16, tag="qkT")
                pair_data[bp] = (qkT, [], xt)
                for jt in range(8):
                    pqk = pod.tile([128, 512], F32, tag="pod", name="pqk")
                    for it in range(4):
                        nc.tensor.matmul(
                            pqk[:],
                            lhsT=wq_sb[:, it, jt * 128 : (jt + 1) * 128],
                            rhs=xt[:, it, :],
                            start=(it == 0),
                            stop=(it == 3),
                        )
                    if jt < o["qk_evac_act"]:
                        nc.scalar.copy(out=qkT[:, jt, :], in_=pqk[:])
                    else:
                        nc.vector.tensor_copy(out=qkT[:, jt, :], in_=pqk[:])
                    yield

            def gen_v(bp):
                """Emit v projection for pair bp (filler during its own
                pair's dots phase); yields between groups."""
                _, vts, xt = pair_data[bp]
                for b in range(2):
                    vt = vtpool.tile([128, 2, 512], BF16, tag="vt")
                    for nt in range(2):
                        pv = pod.tile([128, 512], F32, tag="pod", name="pv")
                        for it in range(4):
                            nc.tensor.matmul(
                                pv[:],
                                lhsT=xt[:, it, b * 256 + nt * 128 : b * 256 + nt * 128 + 128],
                                rhs=wq_sb[:, it, 1024:1536],
                                start=(it == 0),
                                stop=(it == 3),
                            )
                        if o["v_evac_act"]:
                            nc.scalar.copy(out=vt[:, nt, :], in_=pv[:])
                        else:
                            nc.vector.tensor_copy(out=vt[:, nt, :], in_=pv[:])
                        yield
                    vts.append(vt)

            def gen_attn(bp):
                """Emit attention+output for pair bp; yields between groups."""
                qkT, vts, _ = pair_data[bp]
                for b in range(2):
                    # attention scores via K=32 row-packed matmuls, exp, bias
                    attn = [
                        attnpool.tile([128, 4096], BF16, tag=f"attn{mt}",
                                      name=f"attn{mt}")
                        for mt in range(2)
                    ]
                    for g in range(4):
                        for mt in range(2):
                            # 4 row-group matmuls run concurrently on HW and
                            # drain to the SAME partitions — each must target
                            # its own PSUM bank (same-bank row-packed writes
                            # are a fatal PSUM collision). Head hp writes the
                            # first half of bank hp of a 4-bank tile.
                            pd = pdp.tile([128, 2048], F32, tag="pd")
                            for hp in range(4):
                                nc.tensor.matmul(
                                    pd[:, hp * 512 : hp * 512 + 256],
                                    lhsT=qkT[
                                        32 * hp : 32 * (hp + 1),
                                        4 + g,
                                        b * 256 + mt * 128 : b * 256 + mt * 128 + 128,
                                    ],
                                    rhs=qkT[
                                        32 * hp : 32 * (hp + 1),
                                        g,
                                        b * 256 : (b + 1) * 256,
                                    ],
                                    start=True,
                                    stop=True,
                                    tile_position=(32 * hp, 0),
                                    skip_group_check=True,
                                )
                            nc.scalar.activation(
                                out=attn[mt][
                                    :, g * 1024 : (g + 1) * 1024
                                ].rearrange("p (q n) -> p q n", n=256),
                                in_=pd[:].rearrange("p (q n) -> p q n", n=512)[
                                    :, :, 0:256
                                ],
                                func=EXP,
                            )
                            eng = (
                                nc.vector
                                if (2 * g + mt) % 8 < o["eb_dve_mod"]
                                else nc.gpsimd
                            )
                            eng.tensor_mul(
                                attn[mt][:, g * 1024 : (g + 1) * 1024],
                                attn[mt][:, g * 1024 : (g + 1) * 1024],
                                eb_sb[:, mt, g * 1024 : (g + 1) * 1024],
                            )
                            yield "dots"

                    # attn @ v (+ denominators via ones-stationary matmuls).
                    # vts[b] is produced by gen_v filler groups spent during
                    # the dots phase above, so only read it here.
                    vt = vts[b]
                    outT = small.tile([128, 1024], BF16, tag="outT")
                    for g in range(4):
                        od = podd.tile([128, 512], F32, tag="podd")
                        # mt-outer so the four col-group matmuls issue
                        # back-to-back (per-subarray concurrency)
                        for mt in range(2):
                            for hp in range(4):
                                h = 4 * g + hp
                                nc.tensor.matmul(
                                    od[32 * hp : 32 * (hp + 1), 0:256],
                                    lhsT=vt[:, mt, 32 * h : 32 * h + 32],
                                    rhs=attn[mt][:, h * 256 : (h + 1) * 256],
                                    start=(mt == 0),
                                    stop=(mt == 1),
                                    tile_position=(0, 32 * hp),
                                    skip_group_check=True,
                                )
                        for mt in range(2):
                            for hp in range(4):
                                h = 4 * g + hp
                                nc.tensor.matmul(
                                    od[32 * hp : 32 * (hp + 1), 256:512],
                                    lhsT=ones32[:],
                                    rhs=attn[mt][:, h * 256 : (h + 1) * 256],
                                    start=(mt == 0),
                                    stop=(mt == 1),
                                    tile_position=(0, 32 * hp),
                                    skip_group_check=True,
                                )
                        r = small.tile([128, 256], F32, tag="r")
                        nc.vector.reciprocal_approx_fast(out=r[:], in_=od[:, 256:512])
                        nc.vector.tensor_mul(
                            outT[:, g * 256 : (g + 1) * 256], od[:, 0:256], r[:]
                        )
                        yield "attnv"

                    # output projection + bias, PSUM -> SBUF -> DRAM.
                    # Deferred: emitted later as PE filler during the next
                    # batch's ACT-bound dots phase.
                    def emit_proj(b, outT):
                        for nt in range(2):
                            py = pod.tile([128, 512], F32, tag="pod", name="py")
                            for ot in range(4):
                                nc.tensor.matmul(
                                    py[:],
                                    lhsT=outT[
                                        :,
                                        ot * 256 + nt * 128 : ot * 256 + nt * 128 + 128,
                                    ],
                                    rhs=w2_sb[:, ot, :],
                                    start=(ot == 0),
                                    stop=(ot == 3),
                                )
                            ysb = small.tile([128, 512], F32, tag="ysb", name="ysb")
                            nc.vector.tensor_add(ysb[:], py[:], bout_bc[:])
                            nc.sync.dma_start(
                                out=y[(2 * bp + b) % bl, nt], in_=ysb[:]
                            )
                            yield "proj"
                    yield ("proj_gen", emit_proj(b, outT))

            # software pipeline: qkv(pair p+1) groups and deferred output
            # projections are interleaved into the ACT-bound dots phase of
            # attention(pair p) so the PE FIFO never head-of-line blocks on
            # the exp drain.
            total_pairs = repeats * npairs
            for _ in gen_qk(0, xt=xt_pre):
                pass
            for _ in gen_v(0):
                pass
            _SENTINEL = object()
            fillers = []

            def spend_one():
                while fillers:
                    if next(fillers[0], _SENTINEL) is _SENTINEL:
                        fillers.pop(0)
                        continue
                    return True
                return False

            for p in range(total_pairs):
                if p > 0:
                    # v projection of pair p fills its own dots phase (it is
                    # only needed by attnv, after dots); qk of pair p+1 next
                    fillers.append(gen_v(p))
                if p + 1 < total_pairs:
                    fillers.append(gen_qk(p + 1))
                credit = 0.0
                for item in gen_attn(p):
                    if isinstance(item, tuple) and item[0] == "proj_gen":
                        fillers.append(item[1])
                        continue
                    if item == "dots":
                        credit += 1.0
                        while credit >= 1.0 and spend_one():
                            credit -= 1.0
                # flush before the next pipeline stage
                while spend_one():
                    pass
                del pair_data[p]

    nc.compile()
    return nc


def _get_nc(bl: int, repeats: int = 1, opts: dict | None = None):
    key = (bl, repeats, tuple(sorted((opts or {}).items())))
    if key not in _CACHE:
        _CACHE[key] = _build(bl, repeats, opts)
    return _CACHE[key]


def _prep_inputs(x, w_qkv, rel_bias_table, w_out, b_out):
    """Host-side layout prep: transpose/tile/bf16-cast, bias-table gather."""
    x = np.asarray(x, np.float32)
    w_qkv = np.asarray(w_qkv, np.float32).copy()
    rel_bias_table = np.asarray(rel_bias_table, np.float32)
    w_out = np.asarray(w_out, np.float32)
    b_out = np.asarray(b_out, np.float32)

    # fold the attention scale into the q columns of w_qkv
    w_qkv[:, :OUP] *= SCALE

    # xT_dev[bp, p, it, b*256+n] = x[2bp+b, n, it*128+p]
    xT = (
        x.transpose(0, 2, 1)                 # [B, inp, n]
        .reshape(B // 2, 2, 4, 128, N)       # [bp, b, it, p, n]
        .transpose(0, 3, 2, 1, 4)            # [bp, p, it, b, n]
        .reshape(B // 2, 128, 4, 2 * N)
    )
    xT = np.ascontiguousarray(xT).astype(NPBF16)
    # wqkv_dev[p, it, j] = w_qkv[it*128+p, j]
    wqkv_dev = np.ascontiguousarray(
        w_qkv.reshape(4, 128, 3 * OUP).transpose(1, 0, 2)
    ).astype(NPBF16)
    # w2t_dev[p, ot, q] = w_out.T[ot*128+p, q] = w_out[q, ot*128+p]
    w2t_dev = np.ascontiguousarray(
        w_out.T.reshape(4, 128, OUP).transpose(1, 0, 2)
    ).astype(NPBF16)
    # bias[n, m, h]; ebT_dev[p, mt, h*256+n] = exp(bias[n, mt*128+p, h])
    rel_idx = _relative_index(16, 16)
    bias = rel_bias_table[rel_idx].reshape(N, N, H)  # [n, m, h]
    ebT = np.exp(bias.transpose(2, 1, 0))  # [h, m, n]
    ebT_dev = np.ascontiguousarray(
        ebT.reshape(H, 2, 128, N).transpose(2, 1, 0, 3).reshape(128, 2, H * N)
    ).astype(NPBF16)
    bout_dev = b_out.reshape(1, OUP).astype(np.float32)
    return xT, wqkv_dev, w2t_dev, ebT_dev, bout_dev


def kernel(x, w_qkv, rel_bias_table, w_out, b_out, ih, iw):
    assert int(ih) == 16 and int(iw) == 16
    xT, wqkv_dev, w2t_dev, ebT_dev, bout_dev = _prep_inputs(
        x, w_qkv, rel_bias_table, w_out, b_out
    )

    nc = _get_nc(BL)
    npairs = BL // 2
    in_maps = []
    for c in range(NCORES):
        in_maps.append(
            {
                "xT": np.ascontiguousarray(xT[c * npairs : (c + 1) * npairs]),
                "wqkv": wqkv_dev,
                "w2t": w2t_dev,
                "ebT": ebT_dev,
                "bout": bout_dev,
            }
        )

    trace = bool(os.environ.get("BASS_TRACE_KERNEL"))
    if trace:
        try:
            from antenv.axon_hooks import get_axon_ntff_profile_hook  # noqa: F401
        except ImportError:
            trace = False
    res = run_bass_kernel_spmd(nc, in_maps, core_ids=list(range(NCORES)), trace=trace)
    kernel.last_result = res
    if res.exec_time_ns is not None:
        print(f"HW exec time: {res.exec_time_ns} ns")

    y = np.concatenate(
        [r["y"].reshape(BL, N, OUP) for r in res.results], axis=0
    ).astype(np.float32)
    return y


kernel.last_result = None


# revision 6
# speedup vs baseline: 1.0471x; 1.0098x over previous
"""Trainium2 Bass kernel for nn_Attention — v1 rewrite.

Data-parallel over batch across 8 NeuronCores (8 batches/core, processed in
pairs). Per core:
  - qkv projection batch-PAIR weight-stationary: rhs = 2 batches' tokens
    (N=512 streams, halves LDWEIGHTS on HW); q,k come out transposed
    (qkT [j, (b,n)]), v untransposed (v [n, j]).
  - dots^T[m, n] per head via K=32 ROW-PACKED matmuls (tile_position row
    groups): lhsT = k_h^T slice, rhs = q_h^T slice read DIRECTLY from qkT —
    no zero-padded q staging, no SBUF->SBUF DMAs. 4 heads of a group run in
    4 row groups concurrently on HW.
  - softmax without max-subtraction, normalization deferred:
    attn = exp(dots^T) * exp(bias^T) (exp on ACT from PSUM, bias mul on
    DVE/GPSIMD in bf16).
  - attn@v: out_h^T[d, n] = v_h-stationary @ attn^T, 4 heads packed into PE
    col-groups; parallel ones-stationary matmuls give softmax denominators
    as a 32-row broadcast; reciprocal+normalize dense per-partition DVE ops.
  - out projection; bias added via DVE tensor_add from a broadcast tile;
    PSUM -> SBUF -> DRAM.
All matmuls bf16 (fp32 PSUM accumulation).
"""

import os
import sys

import numpy as np

if "/opt/trn_rl_repo" not in sys.path:
    sys.path.insert(0, "/opt/trn_rl_repo")

import ml_dtypes  # noqa: E402

from concourse import bacc, mybir  # noqa: E402
from concourse.tile import TileContext  # noqa: E402
from concourse.bass_utils import run_bass_kernel_spmd  # noqa: E402

BF16 = mybir.dt.bfloat16
F32 = mybir.dt.float32
NPBF16 = ml_dtypes.bfloat16

B, N, INP, OUP, H, D = 64, 256, 512, 512, 16, 32
NCORES = 8
BL = B // NCORES  # batches per core
SCALE = D ** -0.5

_CACHE = {}


def _relative_index(ih: int, iw: int) -> np.ndarray:
    yy, xx = np.meshgrid(np.arange(ih), np.arange(iw), indexing="ij")
    coords = np.stack([yy.ravel(), xx.ravel()])
    rel = coords[:, :, None] - coords[:, None, :]
    rel[0] += ih - 1
    rel[1] += iw - 1
    rel[0] *= 2 * iw - 1
    return rel.sum(0).ravel()


DEFAULT_OPTS = {
    "eb_dve_mod": 0,        # (2g+mt) % 8 < this -> DVE, else GPSIMD
    "pd_bufs": 1,
    "pod_bufs": 2,
    "podd_bufs": 2,
    "xpool_bufs": 3,
    "qkv_bufs": 2,
    "vt_bufs": 4,
    "attn_bufs": 2,
    "small_bufs": 4,
    "qk_evac_act": 0,       # how many of the 8 per-pair qk evacs go to ACT
    "v_evac_act": False,
    "warmup_mms": 8,
}


def _build(bl: int, repeats: int = 1, opts: dict | None = None):
    o = dict(DEFAULT_OPTS)
    if opts:
        o.update(opts)
    nc = bacc.Bacc(None, target_bir_lowering=False)
    npairs = bl // 2

    # xT[bp, p, it, b*256+n] = x[2bp+b, n, it*128+p]
    xT = nc.declare_dram_parameter("xT", [npairs, 128, 4, 512], BF16, isOutput=False)
    wqkv = nc.declare_dram_parameter("wqkv", [128, 4, 1536], BF16, isOutput=False)
    w2t = nc.declare_dram_parameter("w2t", [128, 4, 512], BF16, isOutput=False)
    ebT = nc.declare_dram_parameter("ebT", [128, 2, 4096], BF16, isOutput=False)
    bout = nc.declare_dram_parameter("bout", [1, 512], F32, isOutput=False)
    y = nc.declare_dram_parameter("y", [bl, 2, 128, 512], F32, isOutput=True)

    EXP = mybir.ActivationFunctionType.Exp

    with TileContext(nc) as tc:
        with (
            tc.tile_pool(name="consts", bufs=1) as consts,
            tc.tile_pool(name="xpool", bufs=o["xpool_bufs"]) as xpool,
            tc.tile_pool(name="qkvpool", bufs=o["qkv_bufs"]) as qkvpool,
            tc.tile_pool(name="vtpool", bufs=o["vt_bufs"]) as vtpool,
            tc.tile_pool(name="attnpool", bufs=o["attn_bufs"]) as attnpool,
            tc.tile_pool(name="small", bufs=o["small_bufs"]) as small,
            tc.tile_pool(name="pd", bufs=o["pd_bufs"], space="PSUM") as pdp,
            tc.tile_pool(name="pod", bufs=o["pod_bufs"], space="PSUM") as pod,
            tc.tile_pool(name="podd", bufs=o["podd_bufs"], space="PSUM") as podd,
        ):
            # first x pair on the sync queue ahead of consts so batch 0 can
            # start; qkv weights spread across four engine DMA queues so
            # they all land in parallel
            xt_pre = xpool.tile([128, 4, 512], BF16, tag="xt", name="xt")
            nc.sync.dma_start(xt_pre[:], xT[0])
            wq_sb = consts.tile([128, 4, 1536], BF16)
            nc.scalar.dma_start(wq_sb[:, 0, :], wqkv[:, 0, :])
            nc.gpsimd.dma_start(wq_sb[:, 1, :], wqkv[:, 1, :])
            nc.scalar.dma_start(wq_sb[:, 2, :], wqkv[:, 2, :])
            nc.sync.dma_start(wq_sb[:, 3, :], wqkv[:, 3, :])
            w2_sb = consts.tile([128, 4, 512], BF16)
            nc.scalar.dma_start(w2_sb[:], w2t[:])
            eb_sb = consts.tile([128, 2, 4096], BF16)
            nc.gpsimd.dma_start(eb_sb[:], ebT[:])
            bout_bc = consts.tile([128, 512], F32)
            nc.scalar.dma_start(bout_bc[:], bout[:].to_broadcast((128, 512)))
            ones32 = consts.tile([128, 32], BF16)
            nc.vector.memset(ones32[:], 1.0)
            # warm the PE clock during the initial DMA wait: ~3us of dummy
            # matmuls on a memset tile so the first real matmuls run at
            # full clock (mirrors the HAM warmup window on HW)
            if o["warmup_mms"]:
                wrm = consts.tile([128, 512], BF16)
                nc.vector.memset(wrm[:], 0.0)
                pw = pod.tile([128, 512], F32, tag="pod", name="pwarm")
                for i in range(o["warmup_mms"]):
                    nc.tensor.matmul(
                        pw[:],
                        lhsT=wrm[:, 0:128],
                        rhs=wrm[:],
                        start=(i == 0),
                        stop=(i == o["warmup_mms"] - 1),
                    )

            pair_data = {}

            def gen_qk(bp, xt=None):
                """Emit q,k projection for pair bp; yields between groups."""
                if xt is None:
                    xt = xpool.tile([128, 4, 512], BF16, tag="xt", name="xt")
                    nc.sync.dma_start(xt[:], xT[bp % npairs])
                # q,k projection (transposed), pair-wide: out[j, (b,n)]
                qkT = qkvpool.tile([128, 8, 512], BF16, tag="qkT")
                pair_data[bp] = (qkT, [], xt)
                for jt in range(8):
                    pqk = pod.tile([128, 512], F32, tag="pod", name="pqk")
                    for it in range(4):
                        nc.tensor.matmul(
                            pqk[:],
                            lhsT=wq_sb[:, it, jt * 128 : (jt + 1) * 128],
                            rhs=xt[:, it, :],
                            start=(it == 0),
                            stop=(it == 3),
                        )
                    if jt < o["qk_evac_act"]:
                        nc.scalar.copy(out=qkT[:, jt, :], in_=pqk[:])
                    else:
                        nc.vector.tensor_copy(out=qkT[:, jt, :], in_=pqk[:])
                    yield

            def gen_v(bp):
                """Emit v projection for pair bp (filler during its own
                pair's dots phase); yields between groups."""
                _, vts, xt = pair_data[bp]
                for b in range(2):
                    vt = vtpool.tile([128, 2, 512], BF16, tag="vt")
                    for nt in range(2):
                        pv = pod.tile([128, 512], F32, tag="pod", name="pv")
                        for it in range(4):
                            nc.tensor.matmul(
                                pv[:],
                                lhsT=xt[:, it, b * 256 + nt * 128 : b * 256 + nt * 128 + 128],
                                rhs=wq_sb[:, it, 1024:1536],
                                start=(it == 0),
                                stop=(it == 3),
                            )
                        if o["v_evac_act"]:
                            nc.scalar.copy(out=vt[:, nt, :], in_=pv[:])
                        else:
                            nc.vector.tensor_copy(out=vt[:, nt, :], in_=pv[:])
                        yield
                    vts.append(vt)

            def gen_attn(bp):
                """Emit attention+output for pair bp; yields between groups."""
                qkT, vts, _ = pair_data[bp]
                for b in range(2):
                    # attention scores via K=32 row-packed matmuls, exp, bias
                    attn = [
                        attnpool.tile([128, 4096], BF16, tag=f"attn{mt}",
                                      name=f"attn{mt}")
                        for mt in range(2)
                    ]
                    for gp in range(2):
                        for mt in range(2):
                            # 4 row-group matmuls run concurrently on HW and
                            # drain to the SAME partitions — each must target
                            # its own PSUM bank (same-bank row-packed writes
                            # are a fatal PSUM collision). Two head-GROUPS
                            # (g=2gp, 2gp+1) share the tile: head hp of group
                            # 2gp+gg writes bank hp, half gg. Same-row-group
                            # matmuls (same hp, different gg) serialize on the
                            # PE array cells, so the bank write port never
                            # sees two concurrent drains.
                            pd = pdp.tile([128, 2048], F32, tag="pd")
                            for gg in range(2):
                                g = 2 * gp + gg
                                for hp in range(4):
                                    nc.tensor.matmul(
                                        pd[
                                            :,
                                            hp * 512 + gg * 256 : hp * 512
                                            + (gg + 1) * 256,
                                        ],
                                        lhsT=qkT[
                                            32 * hp : 32 * (hp + 1),
                                            4 + g,
                                            b * 256 + mt * 128 : b * 256
                                            + mt * 128
                                            + 128,
                                        ],
                                        rhs=qkT[
                                            32 * hp : 32 * (hp + 1),
                                            g,
                                            b * 256 : (b + 1) * 256,
                                        ],
                                        start=(gg == 0),
                                        stop=(gg == 1),
                                        tile_position=(32 * hp, 0),
                                        skip_group_check=True,
                                    )
                            # one exp over both groups: in [hp, gg, n] ->
                            # out [gg, hp, n] (attn is [g*1024 + hp*256 + n])
                            nc.scalar.activation(
                                out=attn[mt][
                                    :, gp * 2048 : (gp + 1) * 2048
                                ].rearrange(
                                    "p (gg q n) -> p q gg n", gg=2, n=256
                                ),
                                in_=pd[:].rearrange(
                                    "p (q gg n) -> p q gg n", gg=2, n=256
                                ),
                                func=EXP,
                            )
                            for gg in range(2):
                                g = 2 * gp + gg
                                eng = (
                                    nc.vector
                                    if (2 * g + mt) % 8 < o["eb_dve_mod"]
                                    else nc.gpsimd
                                )
                                eng.tensor_mul(
                                    attn[mt][:, g * 1024 : (g + 1) * 1024],
                                    attn[mt][:, g * 1024 : (g + 1) * 1024],
                                    eb_sb[:, mt, g * 1024 : (g + 1) * 1024],
                                )
                            yield "dots"

                    # attn @ v (+ denominators via ones-stationary matmuls).
                    # vts[b] is produced by gen_v filler groups spent during
                    # the dots phase above, so only read it here.
                    vt = vts[b]
                    outT = small.tile([128, 1024], BF16, tag="outT")
                    for g in range(4):
                        od = podd.tile([128, 512], F32, tag="podd")
                        # mt-outer so the four col-group matmuls issue
                        # back-to-back (per-subarray concurrency)
                        for mt in range(2):
                            for hp in range(4):
                                h = 4 * g + hp
                                nc.tensor.matmul(
                                    od[32 * hp : 32 * (hp + 1), 0:256],
                                    lhsT=vt[:, mt, 32 * h : 32 * h + 32],
                                    rhs=attn[mt][:, h * 256 : (h + 1) * 256],
                                    start=(mt == 0),
                                    stop=(mt == 1),
                                    tile_position=(0, 32 * hp),
                                    skip_group_check=True,
                                )
                        for mt in range(2):
                            for hp in range(4):
                                h = 4 * g + hp
                                nc.tensor.matmul(
                                    od[32 * hp : 32 * (hp + 1), 256:512],
                                    lhsT=ones32[:],
                                    rhs=attn[mt][:, h * 256 : (h + 1) * 256],
                                    start=(mt == 0),
                                    stop=(mt == 1),
                                    tile_position=(0, 32 * hp),
                                    skip_group_check=True,
                                )
                        r = small.tile([128, 256], F32, tag="r")
                        nc.vector.reciprocal_approx_fast(out=r[:], in_=od[:, 256:512])
                        nc.vector.tensor_mul(
                            outT[:, g * 256 : (g + 1) * 256], od[:, 0:256], r[:]
                        )
                        yield "attnv"

                    # output projection + bias, PSUM -> SBUF -> DRAM.
                    # Deferred: emitted later as PE filler during the next
                    # batch's ACT-bound dots phase.
                    def emit_proj(b, outT):
                        for nt in range(2):
                            py = pod.tile([128, 512], F32, tag="pod", name="py")
                            for ot in range(4):
                                nc.tensor.matmul(
                                    py[:],
                                    lhsT=outT[
                                        :,
                                        ot * 256 + nt * 128 : ot * 256 + nt * 128 + 128,
                                    ],
                                    rhs=w2_sb[:, ot, :],
                                    start=(ot == 0),
                                    stop=(ot == 3),
                                )
                            ysb = small.tile([128, 512], F32, tag="ysb", name="ysb")
                            nc.vector.tensor_add(ysb[:], py[:], bout_bc[:])
                            nc.sync.dma_start(
                                out=y[(2 * bp + b) % bl, nt], in_=ysb[:]
                            )
                            yield "proj"
                    yield ("proj_gen", emit_proj(b, outT))

            # software pipeline: qkv(pair p+1) groups and deferred output
            # projections are interleaved into the ACT-bound dots phase of
            # attention(pair p) so the PE FIFO never head-of-line blocks on
            # the exp drain.
            total_pairs = repeats * npairs
            for _ in gen_qk(0, xt=xt_pre):
                pass
            for _ in gen_v(0):
                pass
            _SENTINEL = object()
            fillers = []

            def spend_one():
                while fillers:
                    if next(fillers[0], _SENTINEL) is _SENTINEL:
                        fillers.pop(0)
                        continue
                    return True
                return False

            for p in range(total_pairs):
                if p > 0:
                    # v projection of pair p fills its own dots phase (it is
                    # only needed by attnv, after dots); qk of pair p+1 next
                    fillers.append(gen_v(p))
                if p + 1 < total_pairs:
                    fillers.append(gen_qk(p + 1))
                credit = 0.0
                for item in gen_attn(p):
                    if isinstance(item, tuple) and item[0] == "proj_gen":
                        fillers.append(item[1])
                        continue
                    if item == "dots":
                        # 14 filler groups (4 v + 8 next-pair qk + 2 deferred
                        # proj) per pair, 8 dots yields per pair
                        credit += 1.75
                        while credit >= 1.0 and spend_one():
                            credit -= 1.0
                # flush before the next pipeline stage
                while spend_one():
                    pass
                del pair_data[p]

    nc.compile()
    return nc


def _get_nc(bl: int, repeats: int = 1, opts: dict | None = None):
    key = (bl, repeats, tuple(sorted((opts or {}).items())))
    if key not in _CACHE:
        _CACHE[key] = _build(bl, repeats, opts)
    return _CACHE[key]


def _prep_inputs(x, w_qkv, rel_bias_table, w_out, b_out):
    """Host-side layout prep: transpose/tile/bf16-cast, bias-table gather."""
    x = np.asarray(x, np.float32)
    w_qkv = np.asarray(w_qkv, np.float32).copy()
    rel_bias_table = np.asarray(rel_bias_table, np.float32)
    w_out = np.asarray(w_out, np.float32)
    b_out = np.asarray(b_out, np.float32)

    # fold the attention scale into the q columns of w_qkv
    w_qkv[:, :OUP] *= SCALE

    # xT_dev[bp, p, it, b*256+n] = x[2bp+b, n, it*128+p]
    xT = (
        x.transpose(0, 2, 1)                 # [B, inp, n]
        .reshape(B // 2, 2, 4, 128, N)       # [bp, b, it, p, n]
        .transpose(0, 3, 2, 1, 4)            # [bp, p, it, b, n]
        .reshape(B // 2, 128, 4, 2 * N)
    )
    xT = np.ascontiguousarray(xT).astype(NPBF16)
    # wqkv_dev[p, it, j] = w_qkv[it*128+p, j]
    wqkv_dev = np.ascontiguousarray(
        w_qkv.reshape(4, 128, 3 * OUP).transpose(1, 0, 2)
    ).astype(NPBF16)
    # w2t_dev[p, ot, q] = w_out.T[ot*128+p, q] = w_out[q, ot*128+p]
    w2t_dev = np.ascontiguousarray(
        w_out.T.reshape(4, 128, OUP).transpose(1, 0, 2)
    ).astype(NPBF16)
    # bias[n, m, h]; ebT_dev[p, mt, h*256+n] = exp(bias[n, mt*128+p, h])
    rel_idx = _relative_index(16, 16)
    bias = rel_bias_table[rel_idx].reshape(N, N, H)  # [n, m, h]
    ebT = np.exp(bias.transpose(2, 1, 0))  # [h, m, n]
    ebT_dev = np.ascontiguousarray(
        ebT.reshape(H, 2, 128, N).transpose(2, 1, 0, 3).reshape(128, 2, H * N)
    ).astype(NPBF16)
    bout_dev = b_out.reshape(1, OUP).astype(np.float32)
    return xT, wqkv_dev, w2t_dev, ebT_dev, bout_dev


def kernel(x, w_qkv, rel_bias_table, w_out, b_out, ih, iw):
    assert int(ih) == 16 and int(iw) == 16
    xT, wqkv_dev, w2t_dev, ebT_dev, bout_dev = _prep_inputs(
        x, w_qkv, rel_bias_table, w_out, b_out
    )

    nc = _get_nc(BL)
    npairs = BL // 2
    in_maps = []
    for c in range(NCORES):
        in_maps.append(
            {
                "xT": np.ascontiguousarray(xT[c * npairs : (c + 1) * npairs]),
                "wqkv": wqkv_dev,
                "w2t": w2t_dev,
                "ebT": ebT_dev,
                "bout": bout_dev,
            }
        )

    trace = bool(os.environ.get("BASS_TRACE_KERNEL"))
    if trace:
        try:
            from antenv.axon_hooks import get_axon_ntff_profile_hook  # noqa: F401
        except ImportError:
            trace = False
    res = run_bass_kernel_spmd(nc, in_maps, core_ids=list(range(NCORES)), trace=trace)
    kernel.last_result = res
    if res.exec_time_ns is not None:
        print(f"HW exec time: {res.exec_time_ns} ns")

    y = np.concatenate(
        [r["y"].reshape(BL, N, OUP) for r in res.results], axis=0
    ).astype(np.float32)
    return y


kernel.last_result = None


# revision 7
# speedup vs baseline: 1.0528x; 1.0055x over previous
"""Trainium2 Bass kernel for nn_Attention — v1 rewrite.

Data-parallel over batch across 8 NeuronCores (8 batches/core, processed in
pairs). Per core:
  - qkv projection batch-PAIR weight-stationary: rhs = 2 batches' tokens
    (N=512 streams, halves LDWEIGHTS on HW); q,k come out transposed
    (qkT [j, (b,n)]), v untransposed (v [n, j]).
  - dots^T[m, n] per head via K=32 ROW-PACKED matmuls (tile_position row
    groups): lhsT = k_h^T slice, rhs = q_h^T slice read DIRECTLY from qkT —
    no zero-padded q staging, no SBUF->SBUF DMAs. 4 heads of a group run in
    4 row groups concurrently on HW.
  - softmax without max-subtraction, normalization deferred:
    attn = exp(dots^T) * exp(bias^T) (exp on ACT from PSUM, bias mul on
    DVE/GPSIMD in bf16).
  - attn@v: out_h^T[d, n] = v_h-stationary @ attn^T, 4 heads packed into PE
    col-groups; parallel ones-stationary matmuls give softmax denominators
    as a 32-row broadcast; reciprocal+normalize dense per-partition DVE ops.
  - out projection; bias added via DVE tensor_add from a broadcast tile;
    PSUM -> SBUF -> DRAM.
All matmuls bf16 (fp32 PSUM accumulation).
"""

import os
import sys

import numpy as np

if "/opt/trn_rl_repo" not in sys.path:
    sys.path.insert(0, "/opt/trn_rl_repo")

import ml_dtypes  # noqa: E402

from concourse import bacc, mybir  # noqa: E402
from concourse.tile import TileContext  # noqa: E402
from concourse.bass_utils import run_bass_kernel_spmd  # noqa: E402

BF16 = mybir.dt.bfloat16
F32 = mybir.dt.float32
NPBF16 = ml_dtypes.bfloat16

B, N, INP, OUP, H, D = 64, 256, 512, 512, 16, 32
NCORES = 8
BL = B // NCORES  # batches per core
SCALE = D ** -0.5

_CACHE = {}


def _relative_index(ih: int, iw: int) -> np.ndarray:
    yy, xx = np.meshgrid(np.arange(ih), np.arange(iw), indexing="ij")
    coords = np.stack([yy.ravel(), xx.ravel()])
    rel = coords[:, :, None] - coords[:, None, :]
    rel[0] += ih - 1
    rel[1] += iw - 1
    rel[0] *= 2 * iw - 1
    return rel.sum(0).ravel()


DEFAULT_OPTS = {
    "eb_dve_mod": 0,        # (2g+mt) % 8 < this -> DVE, else GPSIMD
    "pd_bufs": 1,
    "pod_bufs": 3,
    "podd_bufs": 1,
    "xpool_bufs": 3,
    "qkv_bufs": 2,
    "vt_bufs": 4,
    "attn_bufs": 2,
    "small_bufs": 4,
    "qk_evac_act": 0,       # how many of the 8 per-pair qk evacs go to ACT
    "v_evac_act": False,
    "warmup_mms": 8,
}


def _build(bl: int, repeats: int = 1, opts: dict | None = None):
    o = dict(DEFAULT_OPTS)
    if opts:
        o.update(opts)
    nc = bacc.Bacc(None, target_bir_lowering=False)
    npairs = bl // 2

    # xT[bp, p, it, b*256+n] = x[2bp+b, n, it*128+p]
    xT = nc.declare_dram_parameter("xT", [npairs, 128, 4, 512], BF16, isOutput=False)
    wqkv = nc.declare_dram_parameter("wqkv", [128, 4, 1536], BF16, isOutput=False)
    w2t = nc.declare_dram_parameter("w2t", [128, 4, 512], BF16, isOutput=False)
    ebT = nc.declare_dram_parameter("ebT", [128, 2, 4096], BF16, isOutput=False)
    bout = nc.declare_dram_parameter("bout", [1, 512], F32, isOutput=False)
    y = nc.declare_dram_parameter("y", [bl, 2, 128, 512], F32, isOutput=True)

    EXP = mybir.ActivationFunctionType.Exp

    with TileContext(nc) as tc:
        with (
            tc.tile_pool(name="consts", bufs=1) as consts,
            tc.tile_pool(name="xpool", bufs=o["xpool_bufs"]) as xpool,
            tc.tile_pool(name="qkvpool", bufs=o["qkv_bufs"]) as qkvpool,
            tc.tile_pool(name="vtpool", bufs=o["vt_bufs"]) as vtpool,
            tc.tile_pool(name="attnpool", bufs=o["attn_bufs"]) as attnpool,
            tc.tile_pool(name="small", bufs=o["small_bufs"]) as small,
            tc.tile_pool(name="pd", bufs=o["pd_bufs"], space="PSUM") as pdp,
            tc.tile_pool(name="pod", bufs=o["pod_bufs"], space="PSUM") as pod,
            tc.tile_pool(name="podd", bufs=o["podd_bufs"], space="PSUM") as podd,
        ):
            # first x pair on the sync queue ahead of consts so batch 0 can
            # start; qkv weights spread across four engine DMA queues so
            # they all land in parallel
            xt_pre = xpool.tile([128, 4, 512], BF16, tag="xt", name="xt")
            nc.sync.dma_start(xt_pre[:], xT[0])
            wq_sb = consts.tile([128, 4, 1536], BF16)
            nc.scalar.dma_start(wq_sb[:, 0, :], wqkv[:, 0, :])
            nc.gpsimd.dma_start(wq_sb[:, 1, :], wqkv[:, 1, :])
            nc.scalar.dma_start(wq_sb[:, 2, :], wqkv[:, 2, :])
            nc.sync.dma_start(wq_sb[:, 3, :], wqkv[:, 3, :])
            w2_sb = consts.tile([128, 4, 512], BF16)
            nc.scalar.dma_start(w2_sb[:], w2t[:])
            eb_sb = consts.tile([128, 2, 4096], BF16)
            nc.gpsimd.dma_start(eb_sb[:], ebT[:])
            bout_bc = consts.tile([128, 512], F32)
            nc.scalar.dma_start(bout_bc[:], bout[:].to_broadcast((128, 512)))
            ones32 = consts.tile([128, 32], BF16)
            nc.vector.memset(ones32[:], 1.0)
            # warm the PE clock during the initial DMA wait: ~3us of dummy
            # matmuls on a memset tile so the first real matmuls run at
            # full clock (mirrors the HAM warmup window on HW)
            if o["warmup_mms"]:
                wrm = consts.tile([128, 512], BF16)
                nc.vector.memset(wrm[:], 0.0)
                pw = pod.tile([128, 512], F32, tag="pod", name="pwarm")
                for i in range(o["warmup_mms"]):
                    nc.tensor.matmul(
                        pw[:],
                        lhsT=wrm[:, 0:128],
                        rhs=wrm[:],
                        start=(i == 0),
                        stop=(i == o["warmup_mms"] - 1),
                    )

            pair_data = {}

            def gen_qk(bp, xt=None):
                """Emit q,k projection for pair bp; yields between groups."""
                if xt is None:
                    xt = xpool.tile([128, 4, 512], BF16, tag="xt", name="xt")
                    nc.sync.dma_start(xt[:], xT[bp % npairs])
                # q,k projection (transposed), pair-wide: out[j, (b,n)]
                qkT = qkvpool.tile([128, 8, 512], BF16, tag="qkT")
                pair_data[bp] = (qkT, [], xt)
                for jt in range(8):
                    pqk = pod.tile([128, 512], F32, tag="pod", name="pqk")
                    for it in range(4):
                        nc.tensor.matmul(
                            pqk[:],
                            lhsT=wq_sb[:, it, jt * 128 : (jt + 1) * 128],
                            rhs=xt[:, it, :],
                            start=(it == 0),
                            stop=(it == 3),
                        )
                    if jt < o["qk_evac_act"]:
                        nc.scalar.copy(out=qkT[:, jt, :], in_=pqk[:])
                    else:
                        nc.vector.tensor_copy(out=qkT[:, jt, :], in_=pqk[:])
                    yield

            def gen_v(bp):
                """Emit v projection for pair bp (filler during its own
                pair's dots phase); yields between groups."""
                _, vts, xt = pair_data[bp]
                for b in range(2):
                    vt = vtpool.tile([128, 2, 512], BF16, tag="vt")
                    for nt in range(2):
                        pv = pod.tile([128, 512], F32, tag="pod", name="pv")
                        for it in range(4):
                            nc.tensor.matmul(
                                pv[:],
                                lhsT=xt[:, it, b * 256 + nt * 128 : b * 256 + nt * 128 + 128],
                                rhs=wq_sb[:, it, 1024:1536],
                                start=(it == 0),
                                stop=(it == 3),
                            )
                        if o["v_evac_act"]:
                            nc.scalar.copy(out=vt[:, nt, :], in_=pv[:])
                        else:
                            nc.vector.tensor_copy(out=vt[:, nt, :], in_=pv[:])
                        yield
                    vts.append(vt)

            def gen_attn(bp):
                """Emit attention+output for pair bp; yields between groups."""
                qkT, vts, _ = pair_data[bp]
                for b in range(2):
                    # attention scores via K=32 row-packed matmuls, exp, bias
                    attn = [
                        attnpool.tile([128, 4096], BF16, tag=f"attn{mt}",
                                      name=f"attn{mt}")
                        for mt in range(2)
                    ]
                    for gp in range(2):
                        for mt in range(2):
                            # 4 row-group matmuls run concurrently on HW and
                            # drain to the SAME partitions — each must target
                            # its own PSUM bank (same-bank row-packed writes
                            # are a fatal PSUM collision). Two head-GROUPS
                            # (g=2gp, 2gp+1) share the tile: head hp of group
                            # 2gp+gg writes bank hp, half gg. Same-row-group
                            # matmuls (same hp, different gg) serialize on the
                            # PE array cells, so the bank write port never
                            # sees two concurrent drains.
                            pd = pdp.tile([128, 2048], F32, tag="pd")
                            for gg in range(2):
                                g = 2 * gp + gg
                                for hp in range(4):
                                    nc.tensor.matmul(
                                        pd[
                                            :,
                                            hp * 512 + gg * 256 : hp * 512
                                            + (gg + 1) * 256,
                                        ],
                                        lhsT=qkT[
                                            32 * hp : 32 * (hp + 1),
                                            4 + g,
                                            b * 256 + mt * 128 : b * 256
                                            + mt * 128
                                            + 128,
                                        ],
                                        rhs=qkT[
                                            32 * hp : 32 * (hp + 1),
                                            g,
                                            b * 256 : (b + 1) * 256,
                                        ],
                                        start=(gg == 0),
                                        stop=(gg == 1),
                                        tile_position=(32 * hp, 0),
                                        skip_group_check=True,
                                    )
                            # one exp over both groups: in [hp, gg, n] ->
                            # out [gg, hp, n] (attn is [g*1024 + hp*256 + n])
                            nc.scalar.activation(
                                out=attn[mt][
                                    :, gp * 2048 : (gp + 1) * 2048
                                ].rearrange(
                                    "p (gg q n) -> p q gg n", gg=2, n=256
                                ),
                                in_=pd[:].rearrange(
                                    "p (q gg n) -> p q gg n", gg=2, n=256
                                ),
                                func=EXP,
                            )
                            for gg in range(2):
                                g = 2 * gp + gg
                                eng = (
                                    nc.vector
                                    if (2 * g + mt) % 8 < o["eb_dve_mod"]
                                    else nc.gpsimd
                                )
                                eng.tensor_mul(
                                    attn[mt][:, g * 1024 : (g + 1) * 1024],
                                    attn[mt][:, g * 1024 : (g + 1) * 1024],
                                    eb_sb[:, mt, g * 1024 : (g + 1) * 1024],
                                )
                            yield "dots"

                    # attn @ v (+ denominators via ones-stationary matmuls).
                    # vts[b] is produced by gen_v filler groups spent during
                    # the dots phase above, so only read it here.
                    vt = vts[b]
                    outT = small.tile([128, 1024], BF16, tag="outT")
                    for g in range(4):
                        od = podd.tile([128, 512], F32, tag="podd")
                        # mt-outer so the four col-group matmuls issue
                        # back-to-back (per-subarray concurrency)
                        for mt in range(2):
                            for hp in range(4):
                                h = 4 * g + hp
                                nc.tensor.matmul(
                                    od[32 * hp : 32 * (hp + 1), 0:256],
                                    lhsT=vt[:, mt, 32 * h : 32 * h + 32],
                                    rhs=attn[mt][:, h * 256 : (h + 1) * 256],
                                    start=(mt == 0),
                                    stop=(mt == 1),
                                    tile_position=(0, 32 * hp),
                                    skip_group_check=True,
                                )
                        for mt in range(2):
                            for hp in range(4):
                                h = 4 * g + hp
                                nc.tensor.matmul(
                                    od[32 * hp : 32 * (hp + 1), 256:512],
                                    lhsT=ones32[:],
                                    rhs=attn[mt][:, h * 256 : (h + 1) * 256],
                                    start=(mt == 0),
                                    stop=(mt == 1),
                                    tile_position=(0, 32 * hp),
                                    skip_group_check=True,
                                )
                        r = small.tile([128, 256], F32, tag="r")
                        nc.vector.reciprocal_approx_fast(out=r[:], in_=od[:, 256:512])
                        nc.vector.tensor_mul(
                            outT[:, g * 256 : (g + 1) * 256], od[:, 0:256], r[:]
                        )
                        yield "attnv"

                    # output projection + bias, PSUM -> SBUF -> DRAM.
                    # Deferred: emitted later as PE filler during the next
                    # batch's ACT-bound dots phase.
                    def emit_proj(b, outT):
                        for nt in range(2):
                            py = pod.tile([128, 512], F32, tag="pod", name="py")
                            for ot in range(4):
                                nc.tensor.matmul(
                                    py[:],
                                    lhsT=outT[
                                        :,
                                        ot * 256 + nt * 128 : ot * 256 + nt * 128 + 128,
                                    ],
                                    rhs=w2_sb[:, ot, :],
                                    start=(ot == 0),
                                    stop=(ot == 3),
                                )
                            ysb = small.tile([128, 512], F32, tag="ysb", name="ysb")
                            nc.vector.tensor_add(ysb[:], py[:], bout_bc[:])
                            nc.sync.dma_start(
                                out=y[(2 * bp + b) % bl, nt], in_=ysb[:]
                            )
                            yield "proj"
                    yield ("proj_gen", emit_proj(b, outT))

            # software pipeline: qkv(pair p+1) groups and deferred output
            # projections are interleaved into the ACT-bound dots phase of
            # attention(pair p) so the PE FIFO never head-of-line blocks on
            # the exp drain.
            total_pairs = repeats * npairs
            for _ in gen_qk(0, xt=xt_pre):
                pass
            for _ in gen_v(0):
                pass
            _SENTINEL = object()
            fillers = []

            def spend_one():
                while fillers:
                    if next(fillers[0], _SENTINEL) is _SENTINEL:
                        fillers.pop(0)
                        continue
                    return True
                return False

            for p in range(total_pairs):
                if p > 0:
                    # v projection of pair p fills its own dots phase (it is
                    # only needed by attnv, after dots); qk of pair p+1 next
                    fillers.append(gen_v(p))
                if p + 1 < total_pairs:
                    fillers.append(gen_qk(p + 1))
                credit = 0.0
                for item in gen_attn(p):
                    if isinstance(item, tuple) and item[0] == "proj_gen":
                        fillers.append(item[1])
                        continue
                    if item == "dots":
                        # 14 filler groups (4 v + 8 next-pair qk + 2 deferred
                        # proj) per pair, 8 dots yields per pair
                        credit += 1.75
                        while credit >= 1.0 and spend_one():
                            credit -= 1.0
                # flush before the next pipeline stage
                while spend_one():
                    pass
                del pair_data[p]

    nc.compile()
    return nc


def _get_nc(bl: int, repeats: int = 1, opts: dict | None = None):
    key = (bl, repeats, tuple(sorted((opts or {}).items())))
    if key not in _CACHE:
        _CACHE[key] = _build(bl, repeats, opts)
    return _CACHE[key]


def _prep_inputs(x, w_qkv, rel_bias_table, w_out, b_out):
    """Host-side layout prep: transpose/tile/bf16-cast, bias-table gather."""
    x = np.asarray(x, np.float32)
    w_qkv = np.asarray(w_qkv, np.float32).copy()
    rel_bias_table = np.asarray(rel_bias_table, np.float32)
    w_out = np.asarray(w_out, np.float32)
    b_out = np.asarray(b_out, np.float32)

    # fold the attention scale into the q columns of w_qkv
    w_qkv[:, :OUP] *= SCALE

    # xT_dev[bp, p, it, b*256+n] = x[2bp+b, n, it*128+p]
    xT = (
        x.transpose(0, 2, 1)                 # [B, inp, n]
        .reshape(B // 2, 2, 4, 128, N)       # [bp, b, it, p, n]
        .transpose(0, 3, 2, 1, 4)            # [bp, p, it, b, n]
        .reshape(B // 2, 128, 4, 2 * N)
    )
    xT = np.ascontiguousarray(xT).astype(NPBF16)
    # wqkv_dev[p, it, j] = w_qkv[it*128+p, j]
    wqkv_dev = np.ascontiguousarray(
        w_qkv.reshape(4, 128, 3 * OUP).transpose(1, 0, 2)
    ).astype(NPBF16)
    # w2t_dev[p, ot, q] = w_out.T[ot*128+p, q] = w_out[q, ot*128+p]
    w2t_dev = np.ascontiguousarray(
        w_out.T.reshape(4, 128, OUP).transpose(1, 0, 2)
    ).astype(NPBF16)
    # bias[n, m, h]; ebT_dev[p, mt, h*256+n] = exp(bias[n, mt*128+p, h])
    rel_idx = _relative_index(16, 16)
    bias = rel_bias_table[rel_idx].reshape(N, N, H)  # [n, m, h]
    ebT = np.exp(bias.transpose(2, 1, 0))  # [h, m, n]
    ebT_dev = np.ascontiguousarray(
        ebT.reshape(H, 2, 128, N).transpose(2, 1, 0, 3).reshape(128, 2, H * N)
    ).astype(NPBF16)
    bout_dev = b_out.reshape(1, OUP).astype(np.float32)
    return xT, wqkv_dev, w2t_dev, ebT_dev, bout_dev


def kernel(x, w_qkv, rel_bias_table, w_out, b_out, ih, iw):
    assert int(ih) == 16 and int(iw) == 16
    xT, wqkv_dev, w2t_dev, ebT_dev, bout_dev = _prep_inputs(
        x, w_qkv, rel_bias_table, w_out, b_out
    )

    nc = _get_nc(BL)
    npairs = BL // 2
    in_maps = []
    for c in range(NCORES):
        in_maps.append(
            {
                "xT": np.ascontiguousarray(xT[c * npairs : (c + 1) * npairs]),
                "wqkv": wqkv_dev,
                "w2t": w2t_dev,
                "ebT": ebT_dev,
                "bout": bout_dev,
            }
        )

    trace = bool(os.environ.get("BASS_TRACE_KERNEL"))
    if trace:
        try:
            from antenv.axon_hooks import get_axon_ntff_profile_hook  # noqa: F401
        except ImportError:
            trace = False
    res = run_bass_kernel_spmd(nc, in_maps, core_ids=list(range(NCORES)), trace=trace)
    kernel.last_result = res
    if res.exec_time_ns is not None:
        print(f"HW exec time: {res.exec_time_ns} ns")

    y = np.concatenate(
        [r["y"].reshape(BL, N, OUP) for r in res.results], axis=0
    ).astype(np.float32)
    return y


kernel.last_result = None


# revision 8
# speedup vs baseline: 1.0620x; 1.0088x over previous
"""Trainium2 Bass kernel for nn_Attention (dense transformer block).

Data-parallel over batch across 8 NeuronCores (8 batches/core, processed in
pairs). Per core:
  - qkv projection batch-PAIR weight-stationary: rhs = 2 batches' tokens
    (N=512 streams, halves LDWEIGHTS pressure on HW); q,k come out
    transposed (qkT [j, (b,n)]), v untransposed (v [n, j]) so the attn@v
    matmul needs no on-chip transposes.
  - dots^T[m, n] per head via K=32 ROW-PACKED matmuls (tile_position row
    groups): lhsT = k_h^T slice, rhs = q_h^T slice read DIRECTLY from qkT —
    no zero-padded q staging, no SBUF->SBUF DMAs. 4 heads run concurrently
    in 4 row groups on HW; since row-packed matmuls drain to the same
    partitions, each targets its OWN PSUM bank of a 4-bank tile (same-bank
    concurrent row-group writes are a fatal HW PSUM collision that CoreSim
    does not model). Two head-groups share a tile via bank halves —
    same-row-group matmuls serialize on the array cells, so that is safe.
  - softmax without max-subtraction, normalization deferred:
    attn = exp(dots^T) * exp(bias^T), one 2048-wide exp per (gp, mt) on ACT
    reading PSUM; bias multiply on GPSIMD in bf16.
  - attn@v: out_h^T[d, n] = v_h-stationary @ attn^T, 4 heads packed into PE
    col-groups (different partitions -> same-bank writes are safe); parallel
    ones-stationary matmuls give softmax denominators as a 32-row
    broadcast; reciprocal+normalize dense per-partition DVE ops.
  - out projection; bias added via DVE tensor_add from a broadcast tile;
    PSUM -> SBUF -> DRAM.
  - software pipelining by EMISSION order (engine queues are FIFO): ~3us of
    dummy warmup matmuls cover the initial DMA wait (and the HW HAM clock
    warmup window); each pair's v projection and the NEXT pair's q,k
    projection plus the previous batch's deferred output projection are
    interleaved as PE filler into the ACT-bound dots/exp phase.
All matmuls bf16 (fp32 PSUM accumulation); rel-err vs fp32 reference ~3e-3.
"""

import os
import sys

import numpy as np

if "/opt/trn_rl_repo" not in sys.path:
    sys.path.insert(0, "/opt/trn_rl_repo")

import ml_dtypes  # noqa: E402

from concourse import bacc, mybir  # noqa: E402
from concourse.tile import TileContext  # noqa: E402
from concourse.bass_utils import run_bass_kernel_spmd  # noqa: E402

BF16 = mybir.dt.bfloat16
F32 = mybir.dt.float32
NPBF16 = ml_dtypes.bfloat16

B, N, INP, OUP, H, D = 64, 256, 512, 512, 16, 32
NCORES = 8
BL = B // NCORES  # batches per core
SCALE = D ** -0.5

_CACHE = {}


def _relative_index(ih: int, iw: int) -> np.ndarray:
    yy, xx = np.meshgrid(np.arange(ih), np.arange(iw), indexing="ij")
    coords = np.stack([yy.ravel(), xx.ravel()])
    rel = coords[:, :, None] - coords[:, None, :]
    rel[0] += ih - 1
    rel[1] += iw - 1
    rel[0] *= 2 * iw - 1
    return rel.sum(0).ravel()


DEFAULT_OPTS = {
    "eb_dve_mod": 1,        # (2g+mt) % 8 < this -> DVE, else GPSIMD
    "pd_bufs": 1,
    "pod_bufs": 3,
    "podd_bufs": 1,
    "xpool_bufs": 3,
    "qkv_bufs": 2,
    "vt_bufs": 4,
    "attn_bufs": 2,
    "small_bufs": 4,
    "qk_evac_act": 0,       # how many of the 8 per-pair qk evacs go to ACT
    "v_evac_act": False,
    "warmup_mms": 5,
}


def _build(bl: int, repeats: int = 1, opts: dict | None = None):
    o = dict(DEFAULT_OPTS)
    if opts:
        o.update(opts)
    nc = bacc.Bacc(None, target_bir_lowering=False)
    npairs = bl // 2

    # xT[bp, p, it, b*256+n] = x[2bp+b, n, it*128+p]
    xT = nc.declare_dram_parameter("xT", [npairs, 128, 4, 512], BF16, isOutput=False)
    wqkv = nc.declare_dram_parameter("wqkv", [128, 4, 1536], BF16, isOutput=False)
    w2t = nc.declare_dram_parameter("w2t", [128, 4, 512], BF16, isOutput=False)
    ebT = nc.declare_dram_parameter("ebT", [128, 2, 4096], BF16, isOutput=False)
    bout = nc.declare_dram_parameter("bout", [1, 512], F32, isOutput=False)
    y = nc.declare_dram_parameter("y", [bl, 2, 128, 512], F32, isOutput=True)

    EXP = mybir.ActivationFunctionType.Exp

    with TileContext(nc) as tc:
        with (
            tc.tile_pool(name="consts", bufs=1) as consts,
            tc.tile_pool(name="xpool", bufs=o["xpool_bufs"]) as xpool,
            tc.tile_pool(name="qkvpool", bufs=o["qkv_bufs"]) as qkvpool,
            tc.tile_pool(name="vtpool", bufs=o["vt_bufs"]) as vtpool,
            tc.tile_pool(name="attnpool", bufs=o["attn_bufs"]) as attnpool,
            tc.tile_pool(name="small", bufs=o["small_bufs"]) as small,
            tc.tile_pool(name="pd", bufs=o["pd_bufs"], space="PSUM") as pdp,
            tc.tile_pool(name="pod", bufs=o["pod_bufs"], space="PSUM") as pod,
            tc.tile_pool(name="podd", bufs=o["podd_bufs"], space="PSUM") as podd,
        ):
            # first x pair on the sync queue ahead of consts so batch 0 can
            # start; qkv weights spread across four engine DMA queues so
            # they all land in parallel
            xt_pre = xpool.tile([128, 4, 512], BF16, tag="xt", name="xt")
            nc.sync.dma_start(xt_pre[:], xT[0])
            wq_sb = consts.tile([128, 4, 1536], BF16)
            nc.scalar.dma_start(wq_sb[:, 0, :], wqkv[:, 0, :])
            nc.gpsimd.dma_start(wq_sb[:, 1, :], wqkv[:, 1, :])
            nc.scalar.dma_start(wq_sb[:, 2, :], wqkv[:, 2, :])
            nc.sync.dma_start(wq_sb[:, 3, :], wqkv[:, 3, :])
            w2_sb = consts.tile([128, 4, 512], BF16)
            nc.scalar.dma_start(w2_sb[:], w2t[:])
            eb_sb = consts.tile([128, 2, 4096], BF16)
            nc.gpsimd.dma_start(eb_sb[:], ebT[:])
            bout_bc = consts.tile([128, 512], F32)
            nc.scalar.dma_start(bout_bc[:], bout[:].to_broadcast((128, 512)))
            ones32 = consts.tile([128, 32], BF16)
            nc.vector.memset(ones32[:], 1.0)
            # warm the PE clock during the initial DMA wait: ~3us of dummy
            # matmuls on a memset tile so the first real matmuls run at
            # full clock (mirrors the HAM warmup window on HW)
            if o["warmup_mms"]:
                wrm = consts.tile([128, 512], BF16)
                nc.vector.memset(wrm[:], 0.0)
                pw = pod.tile([128, 512], F32, tag="pod", name="pwarm")
                for i in range(o["warmup_mms"]):
                    nc.tensor.matmul(
                        pw[:],
                        lhsT=wrm[:, 0:128],
                        rhs=wrm[:],
                        start=(i == 0),
                        stop=(i == o["warmup_mms"] - 1),
                    )

            pair_data = {}

            def gen_qk(bp, xt=None):
                """Emit q,k projection for pair bp; yields between groups."""
                if xt is None:
                    xt = xpool.tile([128, 4, 512], BF16, tag="xt", name="xt")
                    nc.sync.dma_start(xt[:], xT[bp % npairs])
                # q,k projection (transposed), pair-wide: out[j, (b,n)]
                qkT = qkvpool.tile([128, 8, 512], BF16, tag="qkT")
                pair_data[bp] = (qkT, [], xt)
                for jt in range(8):
                    pqk = pod.tile([128, 512], F32, tag="pod", name="pqk")
                    for it in range(4):
                        nc.tensor.matmul(
                            pqk[:],
                            lhsT=wq_sb[:, it, jt * 128 : (jt + 1) * 128],
                            rhs=xt[:, it, :],
                            start=(it == 0),
                            stop=(it == 3),
                        )
                    if jt < o["qk_evac_act"]:
                        nc.scalar.copy(out=qkT[:, jt, :], in_=pqk[:])
                    else:
                        nc.vector.tensor_copy(out=qkT[:, jt, :], in_=pqk[:])
                    yield

            def gen_v(bp):
                """Emit v projection for pair bp (filler during its own
                pair's dots phase); yields between groups."""
                _, vts, xt = pair_data[bp]
                for b in range(2):
                    vt = vtpool.tile([128, 2, 512], BF16, tag="vt")
                    for nt in range(2):
                        pv = pod.tile([128, 512], F32, tag="pod", name="pv")
                        for it in range(4):
                            nc.tensor.matmul(
                                pv[:],
                                lhsT=xt[:, it, b * 256 + nt * 128 : b * 256 + nt * 128 + 128],
                                rhs=wq_sb[:, it, 1024:1536],
                                start=(it == 0),
                                stop=(it == 3),
                            )
                        if o["v_evac_act"]:
                            nc.scalar.copy(out=vt[:, nt, :], in_=pv[:])
                        else:
                            nc.vector.tensor_copy(out=vt[:, nt, :], in_=pv[:])
                        yield
                    vts.append(vt)

            def gen_attn(bp):
                """Emit attention+output for pair bp; yields between groups."""
                qkT, vts, _ = pair_data[bp]
                for b in range(2):
                    # attention scores via K=32 row-packed matmuls, exp, bias
                    attn = [
                        attnpool.tile([128, 4096], BF16, tag=f"attn{mt}",
                                      name=f"attn{mt}")
                        for mt in range(2)
                    ]
                    for gp in range(2):
                        for mt in range(2):
                            # 4 row-group matmuls run concurrently on HW and
                            # drain to the SAME partitions — each must target
                            # its own PSUM bank (same-bank row-packed writes
                            # are a fatal PSUM collision). Two head-GROUPS
                            # (g=2gp, 2gp+1) share the tile: head hp of group
                            # 2gp+gg writes bank hp, half gg. Same-row-group
                            # matmuls (same hp, different gg) serialize on the
                            # PE array cells, so the bank write port never
                            # sees two concurrent drains.
                            pd = pdp.tile([128, 2048], F32, tag="pd")
                            for gg in range(2):
                                g = 2 * gp + gg
                                for hp in range(4):
                                    nc.tensor.matmul(
                                        pd[
                                            :,
                                            hp * 512 + gg * 256 : hp * 512
                                            + (gg + 1) * 256,
                                        ],
                                        lhsT=qkT[
                                            32 * hp : 32 * (hp + 1),
                                            4 + g,
                                            b * 256 + mt * 128 : b * 256
                                            + mt * 128
                                            + 128,
                                        ],
                                        rhs=qkT[
                                            32 * hp : 32 * (hp + 1),
                                            g,
                                            b * 256 : (b + 1) * 256,
                                        ],
                                        start=(gg == 0),
                                        stop=(gg == 1),
                                        tile_position=(32 * hp, 0),
                                        skip_group_check=True,
                                    )
                            # one exp over both groups: in [hp, gg, n] ->
                            # out [gg, hp, n] (attn is [g*1024 + hp*256 + n])
                            nc.scalar.activation(
                                out=attn[mt][
                                    :, gp * 2048 : (gp + 1) * 2048
                                ].rearrange(
                                    "p (gg q n) -> p q gg n", gg=2, n=256
                                ),
                                in_=pd[:].rearrange(
                                    "p (q gg n) -> p q gg n", gg=2, n=256
                                ),
                                func=EXP,
                            )
                            for gg in range(2):
                                g = 2 * gp + gg
                                eng = (
                                    nc.vector
                                    if (2 * g + mt) % 8 < o["eb_dve_mod"]
                                    else nc.gpsimd
                                )
                                eng.tensor_mul(
                                    attn[mt][:, g * 1024 : (g + 1) * 1024],
                                    attn[mt][:, g * 1024 : (g + 1) * 1024],
                                    eb_sb[:, mt, g * 1024 : (g + 1) * 1024],
                                )
                            yield "dots"

                    # attn @ v (+ denominators via ones-stationary matmuls).
                    # vts[b] is produced by gen_v filler groups spent during
                    # the dots phase above, so only read it here.
                    vt = vts[b]
                    outT = small.tile([128, 1024], BF16, tag="outT")
                    for g in range(4):
                        od = podd.tile([128, 512], F32, tag="podd")
                        # mt-outer so the four col-group matmuls issue
                        # back-to-back (per-subarray concurrency)
                        for mt in range(2):
                            for hp in range(4):
                                h = 4 * g + hp
                                nc.tensor.matmul(
                                    od[32 * hp : 32 * (hp + 1), 0:256],
                                    lhsT=vt[:, mt, 32 * h : 32 * h + 32],
                                    rhs=attn[mt][:, h * 256 : (h + 1) * 256],
                                    start=(mt == 0),
                                    stop=(mt == 1),
                                    tile_position=(0, 32 * hp),
                                    skip_group_check=True,
                                )
                        for mt in range(2):
                            for hp in range(4):
                                h = 4 * g + hp
                                nc.tensor.matmul(
                                    od[32 * hp : 32 * (hp + 1), 256:512],
                                    lhsT=ones32[:],
                                    rhs=attn[mt][:, h * 256 : (h + 1) * 256],
                                    start=(mt == 0),
                                    stop=(mt == 1),
                                    tile_position=(0, 32 * hp),
                                    skip_group_check=True,
                                )
                        r = small.tile([128, 256], F32, tag="r")
                        nc.vector.reciprocal_approx_fast(out=r[:], in_=od[:, 256:512])
                        nc.vector.tensor_mul(
                            outT[:, g * 256 : (g + 1) * 256], od[:, 0:256], r[:]
                        )
                        yield "attnv"

                    # output projection + bias, PSUM -> SBUF -> DRAM.
                    # Deferred: emitted later as PE filler during the next
                    # batch's ACT-bound dots phase.
                    def emit_proj(b, outT):
                        for nt in range(2):
                            py = pod.tile([128, 512], F32, tag="pod", name="py")
                            for ot in range(4):
                                nc.tensor.matmul(
                                    py[:],
                                    lhsT=outT[
                                        :,
                                        ot * 256 + nt * 128 : ot * 256 + nt * 128 + 128,
                                    ],
                                    rhs=w2_sb[:, ot, :],
                                    start=(ot == 0),
                                    stop=(ot == 3),
                                )
                            ysb = small.tile([128, 512], F32, tag="ysb", name="ysb")
                            nc.vector.tensor_add(ysb[:], py[:], bout_bc[:])
                            nc.sync.dma_start(
                                out=y[(2 * bp + b) % bl, nt], in_=ysb[:]
                            )
                            yield "proj"
                    yield ("proj_gen", emit_proj(b, outT))

            # software pipeline: qkv(pair p+1) groups and deferred output
            # projections are interleaved into the ACT-bound dots phase of
            # attention(pair p) so the PE FIFO never head-of-line blocks on
            # the exp drain.
            total_pairs = repeats * npairs
            for _ in gen_qk(0, xt=xt_pre):
                pass
            for _ in gen_v(0):
                pass
            _SENTINEL = object()
            fillers = []

            def spend_one():
                while fillers:
                    if next(fillers[0], _SENTINEL) is _SENTINEL:
                        fillers.pop(0)
                        continue
                    return True
                return False

            for p in range(total_pairs):
                if p > 0:
                    # v projection of pair p fills its own dots phase (it is
                    # only needed by attnv, after dots); qk of pair p+1 next
                    fillers.append(gen_v(p))
                if p + 1 < total_pairs:
                    fillers.append(gen_qk(p + 1))
                credit = 0.0
                for item in gen_attn(p):
                    if isinstance(item, tuple) and item[0] == "proj_gen":
                        fillers.append(item[1])
                        continue
                    if item == "dots":
                        # 14 filler groups (4 v + 8 next-pair qk + 2 deferred
                        # proj) per pair, 8 dots yields per pair
                        credit += 1.75
                        while credit >= 1.0 and spend_one():
                            credit -= 1.0
                # flush before the next pipeline stage
                while spend_one():
                    pass
                del pair_data[p]

    nc.compile()
    return nc


def _get_nc(bl: int, repeats: int = 1, opts: dict | None = None):
    key = (bl, repeats, tuple(sorted((opts or {}).items())))
    if key not in _CACHE:
        _CACHE[key] = _build(bl, repeats, opts)
    return _CACHE[key]


def _prep_inputs(x, w_qkv, rel_bias_table, w_out, b_out):
    """Host-side layout prep: transpose/tile/bf16-cast, bias-table gather."""
    x = np.asarray(x, np.float32)
    w_qkv = np.asarray(w_qkv, np.float32).copy()
    rel_bias_table = np.asarray(rel_bias_table, np.float32)
    w_out = np.asarray(w_out, np.float32)
    b_out = np.asarray(b_out, np.float32)

    # fold the attention scale into the q columns of w_qkv
    w_qkv[:, :OUP] *= SCALE

    # xT_dev[bp, p, it, b*256+n] = x[2bp+b, n, it*128+p]
    xT = (
        x.transpose(0, 2, 1)                 # [B, inp, n]
        .reshape(B // 2, 2, 4, 128, N)       # [bp, b, it, p, n]
        .transpose(0, 3, 2, 1, 4)            # [bp, p, it, b, n]
        .reshape(B // 2, 128, 4, 2 * N)
    )
    xT = np.ascontiguousarray(xT).astype(NPBF16)
    # wqkv_dev[p, it, j] = w_qkv[it*128+p, j]
    wqkv_dev = np.ascontiguousarray(
        w_qkv.reshape(4, 128, 3 * OUP).transpose(1, 0, 2)
    ).astype(NPBF16)
    # w2t_dev[p, ot, q] = w_out.T[ot*128+p, q] = w_out[q, ot*128+p]
    w2t_dev = np.ascontiguousarray(
        w_out.T.reshape(4, 128, OUP).transpose(1, 0, 2)
    ).astype(NPBF16)
    # bias[n, m, h]; ebT_dev[p, mt, h*256+n] = exp(bias[n, mt*128+p, h])
    rel_idx = _relative_index(16, 16)
    bias = rel_bias_table[rel_idx].reshape(N, N, H)  # [n, m, h]
    ebT = np.exp(bias.transpose(2, 1, 0))  # [h, m, n]
    ebT_dev = np.ascontiguousarray(
        ebT.reshape(H, 2, 128, N).transpose(2, 1, 0, 3).reshape(128, 2, H * N)
    ).astype(NPBF16)
    bout_dev = b_out.reshape(1, OUP).astype(np.float32)
    return xT, wqkv_dev, w2t_dev, ebT_dev, bout_dev


def kernel(x, w_qkv, rel_bias_table, w_out, b_out, ih, iw):
    assert int(ih) == 16 and int(iw) == 16
    xT, wqkv_dev, w2t_dev, ebT_dev, bout_dev = _prep_inputs(
        x, w_qkv, rel_bias_table, w_out, b_out
    )

    nc = _get_nc(BL)
    npairs = BL // 2
    in_maps = []
    for c in range(NCORES):
        in_maps.append(
            {
                "xT": np.ascontiguousarray(xT[c * npairs : (c + 1) * npairs]),
                "wqkv": wqkv_dev,
                "w2t": w2t_dev,
                "ebT": ebT_dev,
                "bout": bout_dev,
            }
        )

    trace = bool(os.environ.get("BASS_TRACE_KERNEL"))
    if trace:
        try:
            from antenv.axon_hooks import get_axon_ntff_profile_hook  # noqa: F401
        except ImportError:
            trace = False
    res = run_bass_kernel_spmd(nc, in_maps, core_ids=list(range(NCORES)), trace=trace)
    kernel.last_result = res
    if res.exec_time_ns is not None:
        print(f"HW exec time: {res.exec_time_ns} ns")

    y = np.concatenate(
        [r["y"].reshape(BL, N, OUP) for r in res.results], axis=0
    ).astype(np.float32)
    return y


kernel.last_result = None
